# revision 1
# baseline (speedup 1.0000x reference)
"""Trainium2 Bass kernel for the DSSM (dual-modality Mamba-style 2D selective
scan) module. 8-core SPMD: scan channels d-sharded (24/core x 4 directions),
upstream in_proj/dwconv d-sharded, downstream LN/out position-sharded.
Cross-core: one AllReduce (x_dbl partials + chan-attn MLP partials) and one
AllToAll (y halves channel->position reshard).
"""
import sys
sys.path.insert(0, "/opt/trn_rl_repo")
import numpy as np
import concourse.bass as bass
from concourse import mybir
from concourse.bacc import Bacc
from concourse.tile import TileContext
from concourse.bass_utils import run_bass_kernel_spmd

F32 = mybir.dt.float32
AF = mybir.ActivationFunctionType
OP = mybir.AluOpType

NCORES = 8
RG = [list(range(NCORES))]
B, H, W = 1, 48, 48
HW = H * W                      # 2304
L = 2 * HW                      # 4608
DM = 96                         # d_model
DI = 192                        # d_inner
NST = 4                         # d_state
RNK = 6                         # dt_rank
K = 4
DSL = DI // NCORES              # 24 channels per core
LANES = NST * DSL               # 96 scan lanes (lane = n*DSL + d)
CH = 512                        # phase-B column chunk
NCH = L // CH                   # 9
PC = HW // NCORES               # 288 positions per core (phase C)
RCH = 480                       # phase-A chunk = 10 image rows
ROWCHUNKS = [(0, 10), (10, 10), (20, 10), (30, 10), (40, 8)]
# r1 allreduce buffer: [14, 6*2304 + 4] (xdbl partials | attn v1 partials)
R1C = 6 * HW + 4
HALF_OFF = {(0, 0): 0, (0, 1): HW, (1, 0): 2 * HW, (1, 1): 3 * HW,
            (2, 1): 4 * HW, (3, 1): 5 * HW}  # (k, half)->col offset in r1

_cache = {}


def _build():
    nc = Bacc(trn_type="TRN2", num_devices=NCORES)
    EIn = dict(kind="ExternalInput")
    # per-core inputs (host-prepped)
    i_xvt = nc.dram_tensor("xvt", [DM, HW], F32, **EIn)
    i_xit = nc.dram_tensor("xit", [DM, HW], F32, **EIn)
    i_wxv = nc.dram_tensor("wxv", [DM, DSL], F32, **EIn)    # in_proj xv rows
    i_wzv = nc.dram_tensor("wzv", [DM, DSL], F32, **EIn)    # in_proj zv rows
    i_wxi = nc.dram_tensor("wxi", [DM, DSL], F32, **EIn)
    i_wzi = nc.dram_tensor("wzi", [DM, DSL], F32, **EIn)
    i_wsub = nc.dram_tensor("wsub", [DM, DSL], F32, **EIn)
    i_w9 = nc.dram_tensor("w9", [DSL, 3, 9, DSL], F32, **EIn)  # conv diag/tap/group
    i_b72 = nc.dram_tensor("b72", [DSL, 3], F32, **EIn)        # conv bias per group
    i_wpk = nc.dram_tensor("wpk", [DSL, K, 14], F32, **EIn)    # x_dbl partial lhsT
    i_wdtr = nc.dram_tensor("wdtr", [RNK, K, LANES], F32, **EIn)
    i_dtb = nc.dram_tensor("dtb", [LANES, K], F32, **EIn)
    i_asc = nc.dram_tensor("asc", [LANES, K], F32, **EIn)
    i_rep24 = nc.dram_tensor("rep24", [DSL, LANES], F32, **EIn)
    i_repb = nc.dram_tensor("repb", [NST, LANES], F32, **EIn)
    i_m96 = nc.dram_tensor("m96", [LANES, DSL], F32, **EIn)
    i_diagd = nc.dram_tensor("diagd", [DSL, 2, DSL], F32, **EIn)  # (vi,ir) summed D
    i_f1 = nc.dram_tensor("f1", [DSL, 4, 12], F32, **EIn)   # (via,vim,ira,irm)
    i_f2 = nc.dram_tensor("f2", [12, 2, 2, DM], F32, **EIn)  # (mod, chunk, out96)
    i_lnw = nc.dram_tensor("lnw", [DM, 2, 4], F32, **EIn)    # (chunk, gvi bvi gir bir)
    i_wout = nc.dram_tensor("wout", [DM, 2, DM], F32, **EIn)  # (contract chunk, out)
    i_wz = nc.dram_tensor("wz", [DM, 4, DM], F32, **EIn)     # z lhsT (vi0,vi1,ir0,ir1)
    i_onec = nc.dram_tensor("onec", [DM, 1], F32, **EIn)
    i_oner = nc.dram_tensor("oner", [1, DM], F32, **EIn)
    i_xvc = nc.dram_tensor("xvc", [DM, PC], F32, **EIn)
    i_xic = nc.dram_tensor("xic", [DM, PC], F32, **EIn)
    o_out = nc.dram_tensor("out", [DM, PC], F32, kind="ExternalOutput")
    # collective DRAM buffers
    d_r1i = nc.dram_tensor("d_r1i", [14, R1C], F32)
    d_r1o = nc.dram_tensor("d_r1o", [14, R1C], F32, addr_space="Shared")
    d_a2i = nc.dram_tensor("d_a2i", [NCORES, 2 * DSL, PC], F32)
    d_a2o = nc.dram_tensor("d_a2o", [NCORES, 2 * DSL, PC], F32)

    import contextlib
    with TileContext(nc) as tc, contextlib.ExitStack() as ctx:
        wpool = ctx.enter_context(tc.tile_pool(name="weights", bufs=1))
        big = ctx.enter_context(tc.tile_pool(name="big", bufs=1))

        # ---- load weights ----
        def wtile(shape, src):
            t = wpool.tile(shape, F32, tag=src.name, name="w_" + src.name)
            nc.sync.dma_start(out=t, in_=src[:])
            return t
        t_wxv, t_wzv = wtile([DM, DSL], i_wxv), wtile([DM, DSL], i_wzv)
        t_wxi, t_wzi = wtile([DM, DSL], i_wxi), wtile([DM, DSL], i_wzi)
        t_wsub = wtile([DM, DSL], i_wsub)
        t_w9 = wtile([DSL, 3, 9, DSL], i_w9)
        t_b72 = wtile([DSL, 3], i_b72)
        t_wpk = wtile([DSL, K, 14], i_wpk)
        t_wdtr = wtile([RNK, K, LANES], i_wdtr)
        t_dtb = wtile([LANES, K], i_dtb)
        t_asc = wtile([LANES, K], i_asc)
        t_rep24 = wtile([DSL, LANES], i_rep24)
        t_repb = wtile([NST, LANES], i_repb)
        t_m96 = wtile([LANES, DSL], i_m96)
        t_diagd = wtile([DSL, 2, DSL], i_diagd)
        t_f1 = wtile([DSL, 4, 12], i_f1)
        t_f2 = wtile([12, 2, 2, DM], i_f2)
        t_lnw = wtile([DM, 2, 4], i_lnw)
        t_wout = wtile([DM, 2, DM], i_wout)
        t_wz = wtile([DM, 4, DM], i_wz)
        t_onec = wtile([DM, 1], i_onec)
        t_oner = wtile([1, DM], i_oner)
        t_xvc = wtile([DM, PC], i_xvc)
        t_xic = wtile([DM, PC], i_xic)

        t_xvt = big.tile([DM, HW], F32)
        nc.sync.dma_start(out=t_xvt, in_=i_xvt[:])
        t_xit = big.tile([DM, HW], F32)
        nc.sync.dma_start(out=t_xit, in_=i_xit[:])

        # persistent SBUF
        t_xs = {m: big.tile([DSL, HW], F32, tag=f"xs_{m}", name=f"xs_{m}")
                for m in ("sub", "vi", "ir")}
        t_yvi = big.tile([DSL, HW], F32, tag="yvi")
        t_yir = big.tile([DSL, HW], F32, tag="yir")

        # =========== PHASE A: upstream (d-sharded) ===========
        with tc.tile_pool(name="pa1", bufs=1) as pa1, \
             tc.tile_pool(name="pa", bufs=3) as pa, \
             tc.tile_pool(name="pap", bufs=1, space="PSUM") as pap, \
             tc.tile_pool(name="pav", bufs=2, space="PSUM") as pav:
            t_xdiff = pa1.tile([DM, HW], F32, tag="xdiff")
            nc.vector.tensor_sub(t_xdiff[:], t_xvt[:], t_xit[:])

            pads = {}
            for mname in ("sub", "vi", "ir"):
                pads[mname] = pa1.tile([DSL, 50, 50], F32, tag=f"pad_{mname}",
                                        name=f"pad_{mname}")
                nc.vector.memset(pads[mname][:], 0.0)

            # z-branch (for chan-attn pooling only) + per-modality pooled stats
            t_zacc = pa1.tile([DSL, 2, len(ROWCHUNKS)], F32, tag="zacc")
            t_zc = {}
            for im, (mod, wz_, xt) in enumerate(
                    (("vi", t_wzv, t_xvt), ("ir", t_wzi, t_xit))):
                t_zc[mod] = pa1.tile([DSL, HW], F32, tag=f"zc{mod}", name=f"zc{mod}")
                for ic, (r0, nr) in enumerate(ROWCHUNKS):
                    cols = slice(r0 * W, (r0 + nr) * W)
                    p_z = pap.tile([DSL, RCH], F32, tag="pz")
                    nc.tensor.matmul(p_z[:, :nr * W], wz_[:], xt[:, cols],
                                     start=True, stop=True)
                    nc.scalar.activation(t_zc[mod][:, cols], p_z[:, :nr * W],
                                         AF.Silu, accum_out=t_zacc[:, im, ic:ic + 1])
            t_pool = pa1.tile([DSL, 4], F32, tag="tpool")  # (via,vim,ira,irm)
            nc.vector.tensor_reduce(t_pool[:, 0:1], t_zacc[:, 0, :],
                                    axis=mybir.AxisListType.X, op=OP.add)
            nc.vector.tensor_reduce(t_pool[:, 1:2], t_zc["vi"][:],
                                    axis=mybir.AxisListType.X, op=OP.max)
            nc.vector.tensor_reduce(t_pool[:, 2:3], t_zacc[:, 1, :],
                                    axis=mybir.AxisListType.X, op=OP.add)
            nc.vector.tensor_reduce(t_pool[:, 3:4], t_zc["ir"][:],
                                    axis=mybir.AxisListType.X, op=OP.max)
            # v1 partials [12, 4] -> zero-padded [14, 4]
            t_v1 = pa1.tile([14, 4], F32, tag="tv1")
            nc.vector.memset(t_v1[:], 0.0)
            p_v1 = pav.tile([12, 4], F32, tag="pv1")
            for j in range(4):
                nc.tensor.matmul(p_v1[:, j:j + 1], t_f1[:, j, :], t_pool[:, j:j + 1],
                                 start=True, stop=True)
            nc.scalar.copy(t_v1[0:12, :], p_v1[:])
            nc.sync.dma_start(out=d_r1i[:, 6 * HW:R1C], in_=t_v1[:])

            # x-branch in_proj -> padded conv input
            for g, (wg, xt) in enumerate(
                    (("sub", t_xdiff), ("vi", t_xvt), ("ir", t_xit))):
                wmat = {"sub": t_wsub, "vi": t_wxv, "ir": t_wxi}[wg]
                for (r0, nr) in ROWCHUNKS:
                    cols = slice(r0 * W, (r0 + nr) * W)
                    p_x = pap.tile([DSL, RCH], F32, tag="px")
                    nc.tensor.matmul(p_x[:, :nr * W], wmat[:], xt[:, cols],
                                     start=True, stop=True)
                    nc.scalar.copy(
                        pads[wg][:, 1 + r0:1 + r0 + nr, 1:49],
                        p_x[:, :nr * W].rearrange("p (a b) -> p a b", a=nr))

            # depthwise conv 3x3 (9 diag matmuls per group) + bias + silu
            for g, mod in enumerate(("sub", "vi", "ir")):
                for (r0, nr) in ROWCHUNKS:
                    p_c = pav.tile([DSL, RCH], F32, tag="pconv")
                    for tap in range(9):
                        dy, dx = tap // 3, tap % 3
                        nc.tensor.matmul(
                            p_c[:, :nr * W], t_w9[:, g, tap, :],
                            pads[mod][:, r0 + dy:r0 + dy + nr, dx:dx + 48],
                            start=(tap == 0), stop=(tap == 8))
                    nc.scalar.activation(
                        t_xs[mod][:, r0 * W:(r0 + nr) * W], p_c[:, :nr * W],
                        AF.Silu, bias=t_b72[:, g:g + 1], scale=1.0)

            # x_dbl partials -> r1 buffer (DMA straight from PSUM)
            for (k, half), coff in HALF_OFF.items():
                src = t_xs[("sub", "vi")[half] if k == 0 else
                           ("sub", "ir")[half] if k == 1 else
                           "vi" if k == 2 else "ir"]
                for (r0, nr) in ROWCHUNKS:
                    p_d = pap.tile([14, RCH], F32, tag="pxdbl")
                    nc.tensor.matmul(p_d[:, :nr * W], t_wpk[:, k, :],
                                     src[:, r0 * W:(r0 + nr) * W],
                                     start=True, stop=True)
                    t_xe = pa.tile([14, RCH], F32, tag="txdbl", name="t_xe")
                    nc.scalar.copy(t_xe[:, :nr * W], p_d[:, :nr * W])
                    nc.sync.dma_start(
                        out=d_r1i[:, coff + r0 * W:coff + (r0 + nr) * W],
                        in_=t_xe[:, :nr * W])

        # =========== R1: AllReduce ===========
        nc.gpsimd.collective_compute("AllReduce", OP.add, RG,
                                     ins=[d_r1i[:]], outs=[d_r1o[:]])

        # =========== PHASE B: scan middle ===========
        t_v1o = big.tile([12, 4], F32, tag="v1o")
        nc.sync.dma_start(out=t_v1o, in_=d_r1o[0:12, 6 * HW:6 * HW + 4])

        # tile col-spaces: t0=[k0sub|k0vi] r1-cols 0:4608, t1=[k1sub|k1ir]
        # 4608:9216, t2=[k2vi|k3ir] 9216:13824
        def xs_src(t, col):  # compact xs source for tile t at col (0..4607)
            half = col >= HW
            mod = (("sub", "vi"), ("sub", "ir"), ("vi", "ir"))[t][half]
            return t_xs[mod], col - HW if half else col

        with tc.tile_pool(name="pb", bufs=3) as pb, \
             tc.tile_pool(name="pbp", bufs=1, space="PSUM") as pbp, \
             tc.tile_pool(name="pby", bufs=2, space="PSUM") as pby:
            for t in range(3):
                r1off = t * L
                chunk_order = range(NCH) if t < 2 else range(NCH - 1, -1, -1)
                carry = None
                for c in chunk_order:
                    c0 = c * CH
                    # segment pieces within chunk: (start, end, k) in tile cols
                    k_lo = t if t < 2 else 2
                    k_hi = t if t < 2 else 3
                    if c0 >= HW:
                        pieces = [(c0, c0 + CH, k_hi)]
                    elif c0 + CH <= HW:
                        pieces = [(c0, c0 + CH, k_lo)]
                    else:
                        pieces = [(c0, HW, k_lo), (HW, c0 + CH, k_hi)]
                    rc = slice(r1off + c0, r1off + c0 + CH)

                    t_rR = pb.tile([RNK, CH], F32, tag="rR")
                    nc.sync.dma_start(out=t_rR, in_=d_r1o[0:RNK, rc])
                    t_rB = pb.tile([NST, CH], F32, tag="rB")
                    nc.sync.dma_start(out=t_rB, in_=d_r1o[RNK:RNK + NST, rc])
                    p_dts = pbp.tile([LANES, CH], F32, tag="dts")
                    for (s, e, k) in pieces:
                        nc.tensor.matmul(p_dts[:, s - c0:e - c0], t_wdtr[:, k, :],
                                         t_rR[:, s - c0:e - c0],
                                         start=True, stop=True)
                    t_et = pb.tile([LANES, CH], F32, tag="et")
                    for (s, e, k) in pieces:
                        nc.scalar.activation(t_et[:, s - c0:e - c0],
                                             p_dts[:, s - c0:e - c0], AF.Exp,
                                             bias=t_dtb[:, k:k + 1], scale=1.0)
                    t_delta = pb.tile([LANES, CH], F32, tag="delta")
                    nc.scalar.activation(t_delta[:], t_et[:], AF.Ln,
                                         bias=1.0, scale=1.0)
                    t_u = pb.tile([DSL, CH], F32, tag="u")
                    for (s, e, _k) in pieces:
                        src, sc = xs_src(t, s)
                        nc.vector.tensor_mul(t_u[:, s - c0:e - c0],
                                             t_delta[0:DSL, s - c0:e - c0],
                                             src[:, sc:sc + (e - s)])
                    p_u = pbp.tile([LANES, CH], F32, tag="urep")
                    nc.tensor.matmul(p_u[:], t_rep24[:], t_u[:], start=True, stop=True)
                    p_B = pbp.tile([LANES, CH], F32, tag="brep")
                    nc.tensor.matmul(p_B[:], t_repb[:], t_rB[:],
                                     start=True, stop=True)
                    t_bsb = pb.tile([LANES, CH], F32, tag="bsb")
                    nc.scalar.copy(t_bsb[:], p_B[:])
                    t_b = pb.tile([LANES, CH], F32, tag="b")
                    nc.vector.tensor_mul(t_b[:], p_u[:], t_bsb[:])
                    t_a = pb.tile([LANES, CH], F32, tag="a")
                    for (s, e, k) in pieces:
                        nc.scalar.activation(t_a[:, s - c0:e - c0],
                                             t_delta[:, s - c0:e - c0], AF.Exp,
                                             bias=0.0, scale=t_asc[:, k:k + 1])
                    t_h = pb.tile([LANES, CH], F32, tag="h")
                    if t < 2:
                        init = 0.0 if c == 0 else carry[:, CH - 1:CH]
                        nc.vector.tensor_tensor_scan(t_h[:], t_a[:], t_b[:], init,
                                                     OP.mult, OP.add)
                        carry = t_h
                    else:
                        # reverse scan; pieces processed right-to-left
                        for (s, e, k) in reversed(pieces):
                            sl = slice(s - c0, e - c0)
                            if e == L or e == HW:      # scan-time segment start
                                init = 0.0
                            else:
                                init = carry
                            nc.vector.tensor_tensor_scan(
                                t_h[:, sl][:, ::-1], t_a[:, sl][:, ::-1],
                                t_b[:, sl][:, ::-1], init, OP.mult, OP.add)
                            carry = t_h[:, s - c0:s - c0 + 1]

                    # y: only vi/ir halves feed the output
                    ypieces = [((s if t == 2 else max(s, HW)), e, k)
                               for (s, e, k) in pieces if t == 2 or e > HW]
                    if not ypieces:
                        continue
                    y0 = ypieces[0][0] - c0
                    y1 = ypieces[-1][1] - c0
                    t_rC = pb.tile([NST, CH], F32, tag="rC")
                    nc.sync.dma_start(out=t_rC[:, y0:y1],
                                      in_=d_r1o[RNK + NST:14, rc][:, y0:y1])
                    p_C = pbp.tile([LANES, CH], F32, tag="crep")
                    nc.tensor.matmul(p_C[:, y0:y1], t_repb[:],
                                     t_rC[:, y0:y1], start=True, stop=True)
                    t_hc = pb.tile([LANES, CH], F32, tag="hc")
                    nc.vector.tensor_mul(t_hc[:, y0:y1], t_h[:, y0:y1],
                                         p_C[:, y0:y1])
                    p_y = pby.tile([DSL, CH], F32, tag="y")
                    nc.tensor.matmul(p_y[:, y0:y1], t_m96[:], t_hc[:, y0:y1],
                                     start=True, stop=(t == 2))
                    if t < 2:   # D-skip, combined (D_k + D_{k+2}) on fwd tiles
                        for (s, e, _k) in ypieces:
                            src, sc = xs_src(t, s)
                            nc.tensor.matmul(p_y[:, s - c0:e - c0],
                                             t_diagd[:, t, :], src[:, sc:sc + e - s],
                                             start=False, stop=True)
                    # evacuate/accumulate into y_vi / y_ir
                    for (s, e, _k) in ypieces:
                        sl = slice(s - c0, e - c0)
                        if t == 0:
                            nc.scalar.copy(t_yvi[:, s - HW:e - HW], p_y[:, sl])
                        elif t == 1:
                            nc.scalar.copy(t_yir[:, s - HW:e - HW], p_y[:, sl])
                        elif s < HW:  # t2 k2 -> vi
                            nc.vector.tensor_add(t_yvi[:, s:e], t_yvi[:, s:e],
                                                 p_y[:, sl])
                        else:         # t2 k3 -> ir
                            nc.vector.tensor_add(t_yir[:, s - HW:e - HW],
                                                 t_yir[:, s - HW:e - HW], p_y[:, sl])

        # =========== A2A: reshard y channels -> positions ===========
        for j in range(NCORES):
            nc.sync.dma_start(out=d_a2i[j, 0:DSL, :],
                              in_=t_yvi[:, j * PC:(j + 1) * PC])
            nc.sync.dma_start(out=d_a2i[j, DSL:2 * DSL, :],
                              in_=t_yir[:, j * PC:(j + 1) * PC])
        nc.gpsimd.collective_compute("AllToAll", OP.bypass, RG,
                                     ins=[d_a2i[:]], outs=[d_a2o[:]])

        # =========== PHASE C: LN + gate + out (position-sharded) ===========
        with tc.tile_pool(name="pcq", bufs=2) as pcq, \
             tc.tile_pool(name="pcp", bufs=1, space="PSUM") as pcp:
            # gather y chunks [96, PC] x (2 chunks, 2 mods)
            t_y = {}
            for mod, roff in (("vi", 0), ("ir", DSL)):
                for ck in range(2):
                    ty = pcq.tile([DM, PC], F32, tag=f"y{mod}{ck}", name=f"y{mod}{ck}")
                    for jj in range(4):
                        j = ck * 4 + jj
                        nc.sync.dma_start(out=ty[jj * DSL:(jj + 1) * DSL, :],
                                          in_=d_a2o[j, roff:roff + DSL, :])
                    t_y[(mod, ck)] = ty
            # chan-attn scales s = 1 + sigmoid(f2 @ (relu(va)+relu(vm)))
            t_vr = pcq.tile([12, 4], F32, tag="vr")
            nc.scalar.activation(t_vr[:], t_v1o[:], AF.Relu)
            t_vw = pcq.tile([12, 2], F32, tag="vw")
            nc.vector.tensor_add(t_vw[:, 0:1], t_vr[:, 0:1], t_vr[:, 1:2])
            nc.vector.tensor_add(t_vw[:, 1:2], t_vr[:, 2:3], t_vr[:, 3:4])
            t_s = {}
            for ck in range(2):
                p_ca = pcp.tile([DM, 2], F32, tag="pca")
                for mod_i in range(2):
                    nc.tensor.matmul(p_ca[:, mod_i:mod_i + 1], t_f2[:, mod_i, ck, :],
                                     t_vw[:, mod_i:mod_i + 1], start=True, stop=True)
                t_e = pcq.tile([DM, 2], F32, tag="cae")
                nc.scalar.activation(t_e[:], p_ca[:], AF.Exp, bias=0.0, scale=-1.0)
                nc.vector.tensor_scalar_add(t_e[:], t_e[:], 1.0)
                t_r = pcq.tile([DM, 2], F32, tag=f"car{ck}", name=f"car{ck}")
                nc.vector.reciprocal(t_r[:], t_e[:])          # sigmoid
                nc.vector.tensor_scalar_add(t_r[:], t_r[:], 1.0)  # 1 + sigmoid
                t_s[ck] = t_r
            # z recompute at my positions: z = x @ Wz, silu via exp+recip
            t_z = {}
            for zi, (mod, ck) in enumerate(
                    (("vi", 0), ("vi", 1), ("ir", 0), ("ir", 1))):
                xt = t_xvc if mod == "vi" else t_xic
                p_z = pcp.tile([DM, PC], F32, tag="pz2")
                nc.tensor.matmul(p_z[:], t_wz[:, zi, :], xt[:],
                                 start=True, stop=True)
                t_e = pcq.tile([DM, PC], F32, tag="ze")
                nc.scalar.activation(t_e[:], p_z[:], AF.Exp, bias=0.0, scale=-1.0)
                nc.vector.tensor_scalar_add(t_e[:], t_e[:], 1.0)
                t_r = pcq.tile([DM, PC], F32, tag="zr")
                nc.vector.reciprocal(t_r[:], t_e[:])
                tz = pcq.tile([DM, PC], F32, tag=f"z{zi}", name=f"z{zi}")
                nc.vector.tensor_mul(tz[:], p_z[:], t_r[:])
                t_z[(mod, ck)] = tz
            # LN per modality
            t_fin = {}
            for mod in ("vi", "ir"):
                p_s1 = pcp.tile([1, PC], F32, tag="s1")
                p_s2 = pcp.tile([1, PC], F32, tag="s2")
                for ck in range(2):
                    nc.tensor.matmul(p_s1[:], t_onec[:],
                                     t_y[(mod, ck)][:], start=(ck == 0),
                                     stop=(ck == 1))
                for ck in range(2):
                    t_sq = pcq.tile([DM, PC], F32, tag="sq")
                    nc.scalar.activation(t_sq[:], t_y[(mod, ck)][:], AF.Square)
                    nc.tensor.matmul(p_s2[:], t_onec[:],
                                     t_sq[:], start=(ck == 0), stop=(ck == 1))
                t_mu = pcq.tile([1, PC], F32, tag="mu")
                nc.vector.tensor_scalar_mul(t_mu[:], p_s1[:], 1.0 / DI)
                t_musq = pcq.tile([1, PC], F32, tag="musq")
                nc.vector.tensor_mul(t_musq[:], t_mu[:], t_mu[:])
                t_var = pcq.tile([1, PC], F32, tag="var")
                nc.vector.scalar_tensor_tensor(t_var[:], p_s2[:], 1.0 / DI,
                                               t_musq[:], OP.mult, OP.subtract)
                t_eps = pcq.tile([1, 1], F32, tag="eps")
                nc.vector.memset(t_eps[:], 1e-5)
                t_lnv = pcq.tile([1, PC], F32, tag="lnv")
                nc.scalar.activation(t_lnv[:], t_var[:], AF.Ln, bias=t_eps[:], scale=1.0)
                t_rstd = pcq.tile([1, PC], F32, tag="rstd")
                nc.scalar.activation(t_rstd[:], t_lnv[:], AF.Exp, bias=0.0, scale=-0.5)
                t_mur = pcq.tile([1, PC], F32, tag="mur")
                nc.vector.tensor_mul(t_mur[:], t_mu[:], t_rstd[:])
                p_q = pcp.tile([DM, PC], F32, tag="pq")
                nc.tensor.matmul(p_q[:], t_oner[:], t_rstd[:], start=True, stop=True)
                p_m = pcp.tile([DM, PC], F32, tag="pm")
                nc.tensor.matmul(p_m[:], t_oner[:], t_mur[:], start=True, stop=True)
                gb = {"vi": (0, 1), "ir": (2, 3)}[mod]
                for ck in range(2):
                    t_t = pcq.tile([DM, PC], F32, tag="lt")
                    nc.vector.tensor_mul(t_t[:], t_y[(mod, ck)][:], p_q[:])
                    t_t2 = pcq.tile([DM, PC], F32, tag="lt2")
                    nc.vector.tensor_sub(t_t2[:], t_t[:], p_m[:])
                    t_yn = pcq.tile([DM, PC], F32, tag="yn")
                    nc.scalar.activation(t_yn[:], t_t2[:], AF.Identity,
                                         bias=t_lnw[:, ck, gb[1]:gb[1] + 1],
                                         scale=t_lnw[:, ck, gb[0]:gb[0] + 1])
                    # gate: fin += yn * z * s
                    t_m1 = pcq.tile([DM, PC], F32, tag="m1")
                    nc.vector.tensor_mul(t_m1[:], t_yn[:], t_z[(mod, ck)][:])
                    if mod == "vi":
                        t_f = pcq.tile([DM, PC], F32, tag=f"fin{ck}", name=f"fin{ck}")
                        nc.vector.tensor_scalar_mul(t_f[:], t_m1[:],
                                                    t_s[ck][:, 0:1])
                        t_fin[ck] = t_f
                    else:
                        nc.vector.scalar_tensor_tensor(t_fin[ck][:], t_m1[:],
                                                       t_s[ck][:, 1:2], t_fin[ck][:],
                                                       OP.mult, OP.add)
            p_o = pcp.tile([DM, PC], F32, tag="po")
            for ck in range(2):
                nc.tensor.matmul(p_o[:], t_wout[:, ck, :], t_fin[ck][:],
                                 start=(ck == 0), stop=(ck == 1))
            t_o = pcq.tile([DM, PC], F32, tag="o")
            nc.scalar.copy(t_o[:], p_o[:])
            nc.sync.dma_start(out=o_out[:], in_=t_o[:])

    nc.finalize()
    return nc


def _prep_inputs(inputs):
    """Host-side prep: slice/transpose weights per core. Returns in_maps."""
    g = {k: np.asarray(v, dtype=np.float32) for k, v in inputs.items()}
    x_vi = g["x_vi"].reshape(HW, DM)
    x_ir = g["x_ir"].reshape(HW, DM)
    xvt = np.ascontiguousarray(x_vi.T)
    xit = np.ascontiguousarray(x_ir.T)
    A = -np.exp(g["A_logs"]).reshape(K, DI, NST)
    Ds = g["Ds"].reshape(K, DI)
    in_maps = []
    for c in range(NCORES):
        S = slice(c * DSL, (c + 1) * DSL)
        m = {}
        m["xvt"], m["xit"] = xvt, xit
        m["wxv"] = np.ascontiguousarray(g["W_vi"][S].T)
        m["wzv"] = np.ascontiguousarray(g["W_vi"][DI:][S].T)
        m["wxi"] = np.ascontiguousarray(g["W_ir"][S].T)
        m["wzi"] = np.ascontiguousarray(g["W_ir"][DI:][S].T)
        m["wsub"] = np.ascontiguousarray(g["W_sub"][S].T)
        w9 = np.zeros((DSL, 3, 9, DSL), np.float32)
        b72 = np.zeros((DSL, 3), np.float32)
        for gi, nm in enumerate(("sub", "vi", "ir")):
            cw = g[f"conv_w_{nm}"][S, 0]      # [DSL, 3, 3]
            for tap in range(9):
                for d in range(DSL):
                    w9[d, gi, tap, d] = cw[d, tap // 3, tap % 3]
            b72[:, gi] = g[f"conv_b_{nm}"][S]
        m["w9"], m["b72"] = w9, b72
        m["wpk"] = np.ascontiguousarray(
            g["x_proj_weight"][:, :, S].transpose(2, 0, 1))  # [DSL, K, 14]
        wdtr = np.zeros((RNK, K, LANES), np.float32)
        dtb = np.zeros((LANES, K), np.float32)
        asc = np.zeros((LANES, K), np.float32)
        for k in range(K):
            for n in range(NST):
                for d in range(DSL):
                    lane = n * DSL + d
                    wdtr[:, k, lane] = g["dt_projs_weight"][k, c * DSL + d, :]
                    dtb[lane, k] = g["dt_projs_bias"][k, c * DSL + d]
                    asc[lane, k] = A[k, c * DSL + d, n]
        m["wdtr"], m["dtb"], m["asc"] = wdtr, dtb, asc
        rep24 = np.zeros((DSL, LANES), np.float32)
        repb = np.zeros((NST, LANES), np.float32)
        m96 = np.zeros((LANES, DSL), np.float32)
        for n in range(NST):
            for d in range(DSL):
                rep24[d, n * DSL + d] = 1
                repb[n, n * DSL + d] = 1
                m96[n * DSL + d, d] = 1
        m["rep24"], m["repb"], m["m96"] = rep24, repb, m96
        diagd = np.zeros((DSL, 2, DSL), np.float32)
        np.fill_diagonal(diagd[:, 0, :], Ds[0, S] + Ds[2, S])
        np.fill_diagonal(diagd[:, 1, :], Ds[1, S] + Ds[3, S])
        m["diagd"] = diagd
        f1 = np.zeros((DSL, 4, 12), np.float32)
        f1[:, 0] = g["ca_vi_f1"][:, S].T / HW
        f1[:, 1] = g["ca_vi_f1"][:, S].T
        f1[:, 2] = g["ca_ir_f1"][:, S].T / HW
        f1[:, 3] = g["ca_ir_f1"][:, S].T
        m["f1"] = f1
        f2 = np.zeros((12, 2, 2, DM), np.float32)
        for ck in range(2):
            f2[:, 0, ck] = g["ca_vi_f2"][ck * DM:(ck + 1) * DM].T
            f2[:, 1, ck] = g["ca_ir_f2"][ck * DM:(ck + 1) * DM].T
        m["f2"] = f2
        lnw = np.zeros((DM, 2, 4), np.float32)
        for ck in range(2):
            cs = slice(ck * DM, (ck + 1) * DM)
            lnw[:, ck, 0] = g["ln_vi_g"][cs]
            lnw[:, ck, 1] = g["ln_vi_b"][cs]
            lnw[:, ck, 2] = g["ln_ir_g"][cs]
            lnw[:, ck, 3] = g["ln_ir_b"][cs]
        m["lnw"] = lnw
        wout = np.zeros((DM, 2, DM), np.float32)
        for ck in range(2):
            wout[:, ck] = g["W_out"][:, ck * DM:(ck + 1) * DM].T
        m["wout"] = wout
        wz = np.zeros((DM, 4, DM), np.float32)
        wz[:, 0] = g["W_vi"][DI:][0:DM].T
        wz[:, 1] = g["W_vi"][DI:][DM:DI].T
        wz[:, 2] = g["W_ir"][DI:][0:DM].T
        wz[:, 3] = g["W_ir"][DI:][DM:DI].T
        m["wz"] = wz
        m["onec"] = np.ones((DM, 1), np.float32)
        m["oner"] = np.ones((1, DM), np.float32)
        m["xvc"] = np.ascontiguousarray(xvt[:, c * PC:(c + 1) * PC])
        m["xic"] = np.ascontiguousarray(xit[:, c * PC:(c + 1) * PC])
        in_maps.append(m)
    return in_maps


def kernel(**inputs):
    if "nc" not in _cache:
        _cache["nc"] = _build()
    nc = _cache["nc"]
    in_maps = _prep_inputs(inputs)
    res = run_bass_kernel_spmd(nc, in_maps, core_ids=list(range(NCORES)))
    out = np.zeros((DM, HW), np.float32)
    for c in range(NCORES):
        out[:, c * PC:(c + 1) * PC] = res.results[c]["out"]
    return out.T.reshape(B, H, W, DM).astype(np.float32)



# revision 10
# speedup vs baseline: 2.5903x; 2.5903x over previous
"""Trainium2 Bass kernel for the DSSM (dual-modality Mamba-style 2D selective
scan) module. 8-core SPMD: scan channels d-sharded (24/core x 4 directions),
upstream in_proj/dwconv d-sharded with modalities packed into partitions,
downstream LN/out position-sharded. Cross-core: 3 column-sliced AllReduces of
x_dbl partials (pipelined against phase A) and one AllToAll (y reshard).
All wide matmuls run as float32r (full-rate fp32 on the PE array).
"""
import sys
sys.path.insert(0, "/opt/trn_rl_repo")
import numpy as np
import concourse.bass as bass
from concourse import mybir
from concourse.bacc import Bacc
from concourse.tile import TileContext
from concourse.bass_utils import run_bass_kernel_spmd

F32 = mybir.dt.float32
R32 = mybir.dt.float32r
AF = mybir.ActivationFunctionType
OP = mybir.AluOpType

NCORES = 8
RG = [list(range(NCORES))]
B, H, W = 1, 48, 48
HW = H * W                      # 2304
L = 2 * HW                      # 4608
DM = 96                         # d_model
DI = 192                        # d_inner
NST = 4                         # d_state
RNK = 6                         # dt_rank
K = 4
DSL = DI // NCORES              # 24 channels per core
LANES = NST * DSL               # 96 scan lanes (lane = n*DSL + d)
CH = 512                        # phase-B column chunk
NCH = L // CH                   # 9
PC = HW // NCORES               # 288 positions per core (phase C)
RCH = 480                       # phase-A chunk = 10 image rows
ROWCHUNKS = [(0, 10), (10, 10), (20, 10), (30, 10), (40, 8)]
# r1 layout: [84 rows, spatial cols] split in 3 col slices (pipelined AR).
# rows: 6 groups x 14 (dts 0:6 | B 6:10 | C 10:14), group order:
# (k0,sub) (k0,vi) (k1,sub) (k1,ir) (k2,vi) (k3,ir)
GR = 84
SLC = 960                       # r1 col-slice width (2 rowchunks)
SL_COLS = [960, 960, 388]       # slice2: 384 spatial + 4 attn cols
ROWBASE = {(0, 0): 0, (0, 1): 14, (1, 0): 28, (1, 1): 42,
           (2, 0): 56, (2, 1): 70}  # (tile, half) -> r1 row group base
MODOFF = {"sub": 0, "vi": 32, "ir": 64}  # 32-aligned partition blocks

_cache = {}


def _patch_act_tables():
    import concourse.bacc as _bacc
    from concourse.hw_specs import get_activation_tables as _gat
    if getattr(_bacc, "_act_tables_patched", False):
        return
    def patched(arch):
        tabs = {k: set(v) for k, v in _gat(arch).items()}
        # Force exp/ln to resolve to the combined natural_log_exp table so
        # softplus chains (exp -> ln -> exp) never reload act tables.
        for name in ("exp_and_others", "exp_and_friends"):
            if name in tabs:
                tabs[name].discard(AF.Exp)
        if "natural_log" in tabs:
            tabs["natural_log"].discard(AF.Ln)
        return tabs
    _bacc.get_activation_tables = patched
    _bacc._act_tables_patched = True


def _build():
    _patch_act_tables()
    nc = Bacc(trn_type="TRN2", num_devices=NCORES)
    EIn = dict(kind="ExternalInput")
    # per-core inputs (host-prepped)
    i_xvt = nc.dram_tensor("xvt", [DM, HW], F32, **EIn)
    i_xit = nc.dram_tensor("xit", [DM, HW], F32, **EIn)
    i_wxA = nc.dram_tensor("wxA", [DM, 96], F32, **EIn)  # x in_proj vs xvt
    i_wxB = nc.dram_tensor("wxB", [DM, 96], F32, **EIn)  # x in_proj vs xit
    i_wzA = nc.dram_tensor("wzA", [DM, 48], F32, **EIn)  # z in_proj vs xvt
    i_wzB = nc.dram_tensor("wzB", [DM, 48], F32, **EIn)  # z in_proj vs xit
    i_w9p = nc.dram_tensor("w9p", [96, 9, 96], F32, **EIn)  # conv diag, mods packed
    i_b72p = nc.dram_tensor("b72p", [96, 1], F32, **EIn)
    i_wpk84 = nc.dram_tensor("wpk84", [96, GR], F32, **EIn)  # x_dbl block lhsT
    i_wdtr = nc.dram_tensor("wdtr", [RNK, K, LANES], F32, **EIn)
    i_dtb = nc.dram_tensor("dtb", [LANES, K], F32, **EIn)
    i_asc = nc.dram_tensor("asc", [LANES, K], F32, **EIn)
    i_rep24 = nc.dram_tensor("rep24", [DSL, LANES], F32, **EIn)
    i_repb = nc.dram_tensor("repb", [NST, LANES], F32, **EIn)
    i_m96 = nc.dram_tensor("m96", [LANES, DSL], F32, **EIn)
    i_dvec = nc.dram_tensor("dvec", [DSL, 2], F32, **EIn)  # (vi,ir) summed D
    i_f1q = nc.dram_tensor("f1q", [48, 2, 12], F32, **EIn)  # attn mlp1 per mod
    i_f2 = nc.dram_tensor("f2", [12, 2, 2, DM], F32, **EIn)  # (mod, chunk, out96)
    i_lnw = nc.dram_tensor("lnw", [DM, 2, 4], F32, **EIn)    # (chunk, gvi bvi gir bir)
    i_wout = nc.dram_tensor("wout", [DM, 2, DM], F32, **EIn)  # (contract chunk, out)
    i_wz = nc.dram_tensor("wz", [DM, 4, DM], F32, **EIn)     # z lhsT (vi0,vi1,ir0,ir1)
    i_onec = nc.dram_tensor("onec", [DM, 1], F32, **EIn)
    i_oner = nc.dram_tensor("oner", [1, DM], F32, **EIn)
    i_xvc = nc.dram_tensor("xvc", [DM, PC], F32, **EIn)
    i_xic = nc.dram_tensor("xic", [DM, PC], F32, **EIn)
    o_out = nc.dram_tensor("out", [DM, PC], F32, kind="ExternalOutput")
    # collective DRAM buffers (one pair per r1 col-slice)
    d_r1i = [nc.dram_tensor(f"d_r1i{j}", [GR, SL_COLS[j]], F32)
             for j in range(3)]
    d_r1o = [nc.dram_tensor(f"d_r1o{j}", [GR, SL_COLS[j]], F32,
                            addr_space="Shared") for j in range(3)]
    d_a2i = nc.dram_tensor("d_a2i", [NCORES, 2 * DSL, PC], F32)
    d_a2o = nc.dram_tensor("d_a2o", [NCORES, 2 * DSL, PC], F32)

    import contextlib
    with TileContext(nc) as tc, contextlib.ExitStack() as ctx:
        wpool = ctx.enter_context(tc.tile_pool(name="weights", bufs=1))
        big = ctx.enter_context(tc.tile_pool(name="big", bufs=1))

        # ---- load weights ----
        def wtile(shape, src, dt=F32):
            t = wpool.tile(shape, dt, tag=src.name, name="w_" + src.name)
            nc.sync.dma_start(out=t, in_=src[:].bitcast(dt) if dt is R32
                              else src[:])
            return t
        t_wxA = wtile([DM, 96], i_wxA, R32)
        t_wxB = wtile([DM, 96], i_wxB, R32)
        t_wzA = wtile([DM, 48], i_wzA, R32)
        t_wzB = wtile([DM, 48], i_wzB, R32)
        t_w9p = wtile([96, 9, 96], i_w9p, R32)
        t_b72p = wtile([96, 1], i_b72p)
        t_wpk84 = wtile([96, GR], i_wpk84, R32)
        t_wdtr = wtile([RNK, K, LANES], i_wdtr, R32)
        t_dtb = wtile([LANES, K], i_dtb)
        t_asc = wtile([LANES, K], i_asc)
        t_rep24 = wtile([DSL, LANES], i_rep24, R32)
        t_repb = wtile([NST, LANES], i_repb, R32)
        t_m96 = wtile([LANES, DSL], i_m96, R32)
        t_dvec = wtile([DSL, 2], i_dvec)
        t_f1q = wtile([48, 2, 12], i_f1q)
        t_f2 = wtile([12, 2, 2, DM], i_f2)
        t_lnw = wtile([DM, 2, 4], i_lnw)
        t_wout = wtile([DM, 2, DM], i_wout, R32)
        t_wz = wtile([DM, 4, DM], i_wz, R32)
        t_onec = wtile([DM, 1], i_onec, R32)
        t_oner = wtile([1, DM], i_oner, R32)
        t_xvc = wtile([DM, PC], i_xvc, R32)
        t_xic = wtile([DM, PC], i_xic, R32)

        # sliced input loads (per rowchunk) so phase A starts early
        t_xvt = big.tile([DM, HW], R32)
        t_xit = big.tile([DM, HW], R32)
        for (r0, nr) in ROWCHUNKS:
            cs = slice(r0 * W, (r0 + nr) * W)
            nc.sync.dma_start(out=t_xvt[:, cs], in_=i_xvt[:, cs].bitcast(R32))
            nc.sync.dma_start(out=t_xit[:, cs], in_=i_xit[:, cs].bitcast(R32))

        # persistent SBUF
        t_xs96 = big.tile([96, HW], R32, tag="xs96")  # rows: sub@0|vi@32|ir@64
        t_xv24 = big.tile([DSL, HW], R32, tag="xv24")  # base-0 copy of vi block
        t_xi24 = big.tile([DSL, HW], R32, tag="xi24")  # base-0 copy of ir block
        t_yvi = big.tile([DSL, HW], F32, tag="yvi")
        t_yir = big.tile([DSL, HW], F32, tag="yir")

        # =========== PHASE A: upstream (d-sharded, mods packed) ===========
        pa1 = ctx.enter_context(tc.tile_pool(name="pa1", bufs=1))
        with tc.tile_pool(name="pain", bufs=2, space="PSUM") as pain, \
             tc.tile_pool(name="painz", bufs=2, space="PSUM") as painz, \
             tc.tile_pool(name="pacv", bufs=2, space="PSUM") as pacv, \
             tc.tile_pool(name="padb", bufs=2, space="PSUM") as padb:
            t_pad = pa1.tile([96, 50, 50], R32, tag="pad")
            nc.vector.memset(t_pad[:].bitcast(F32), 0.0)

            t_zc = pa1.tile([48, HW], F32, tag="zc")  # silu(z), vi|ir packed
            t_zacc = pa1.tile([48, len(ROWCHUNKS)], F32, tag="zacc")
            # in_proj (+z) packed: x=[sub|vi|ir], z=[zv|zi]
            for ic, (r0, nr) in enumerate(ROWCHUNKS):
                cols = slice(r0 * W, (r0 + nr) * W)
                p_x = pain.tile([96, RCH], F32, tag="pin")
                nc.tensor.matmul(p_x[:, :nr * W], t_wxA[:], t_xvt[:, cols],
                                 start=True, stop=False)
                nc.tensor.matmul(p_x[:, :nr * W], t_wxB[:], t_xit[:, cols],
                                 start=False, stop=True)
                p_z = painz.tile([48, RCH], F32, tag="pz")
                nc.tensor.matmul(p_z[:, :nr * W], t_wzA[:], t_xvt[:, cols],
                                 start=True, stop=False)
                nc.tensor.matmul(p_z[:, :nr * W], t_wzB[:], t_xit[:, cols],
                                 start=False, stop=True)
                nc.scalar.activation(t_zc[:, cols], p_z[:, :nr * W],
                                     AF.Silu, accum_out=t_zacc[:, ic:ic + 1])
                nc.scalar.copy(
                    t_pad[:, 1 + r0:1 + r0 + nr, 1:49],
                    p_x[:, :nr * W].rearrange("p (a b) -> p a b", a=nr))

            # depthwise conv 3x3 (9 block-diag matmuls) + bias + silu -> xs
            for (r0, nr) in ROWCHUNKS:
                p_c = pacv.tile([96, RCH], F32, tag="pconv")
                for tap in range(9):
                    dy, dx = tap // 3, tap % 3
                    nc.tensor.matmul(
                        p_c[:, :nr * W], t_w9p[:, tap, :],
                        t_pad[:, r0 + dy:r0 + dy + nr, dx:dx + 48],
                        start=(tap == 0), stop=(tap == 8))
                cols = slice(r0 * W, (r0 + nr) * W)
                nc.scalar.activation(
                    t_xs96[:, cols], p_c[:, :nr * W],
                    AF.Silu, bias=t_b72p[:], scale=1.0)
                nc.vector.tensor_copy(t_xv24[:, cols], t_xs96[32:56, cols])
                nc.vector.tensor_copy(t_xi24[:, cols], t_xs96[64:88, cols])

            # x_dbl all 6 groups in one matmul per rowchunk -> r1 slices
            for ic, (r0, nr) in enumerate(ROWCHUNKS):
                p_d = padb.tile([GR, RCH], F32, tag="pxdbl")
                nc.tensor.matmul(p_d[:, :nr * W], t_wpk84[:],
                                 t_xs96[:, r0 * W:(r0 + nr) * W],
                                 start=True, stop=True)
                t_xe = pa1.tile([GR, RCH], F32, tag=f"txe{ic}", name=f"txe{ic}")
                nc.scalar.copy(t_xe[:, :nr * W], p_d[:, :nr * W])
                j = (r0 * W) // SLC
                lc = r0 * W - j * SLC
                nc.sync.dma_start(out=d_r1i[j][:, lc:lc + nr * W],
                                  in_=t_xe[:, :nr * W])
                if ic == 1:
                    nc.gpsimd.collective_compute(
                        "AllReduce", OP.add, RG,
                        ins=[d_r1i[0][:]], outs=[d_r1o[0][:]])
                elif ic == 3:
                    nc.gpsimd.collective_compute(
                        "AllReduce", OP.add, RG,
                        ins=[d_r1i[1][:]], outs=[d_r1o[1][:]])
        # chan-attn pooled stats + v1 partials [12, 4] (own PSUM scope)
        with tc.tile_pool(name="pav1", bufs=1, space="PSUM") as pav1:
            t_pool = pa1.tile([48, 2], F32, tag="tpool")  # (avg, max)
            nc.vector.tensor_reduce(t_pool[:, 0:1], t_zacc[:],
                                    axis=mybir.AxisListType.X, op=OP.add)
            nc.vector.tensor_scalar_mul(t_pool[:, 0:1], t_pool[:, 0:1], 1.0 / HW)
            nc.vector.tensor_reduce(t_pool[:, 1:2], t_zc[:],
                                    axis=mybir.AxisListType.X, op=OP.max)
            t_v1 = pa1.tile([12, 4], F32, tag="tv1")  # (via, vim, ira, irm)
            p_v1 = pav1.tile([12, 4], F32, tag="pv1")
            for mi in range(2):
                for st in range(2):
                    nc.tensor.matmul(p_v1[:, 2 * mi + st:2 * mi + st + 1],
                                     t_f1q[:, mi, :], t_pool[:, st:st + 1],
                                     start=True, stop=True)
            nc.scalar.copy(t_v1[:], p_v1[:])
            nc.sync.dma_start(out=d_r1i[2][0:12, 384:388], in_=t_v1[:])
        nc.gpsimd.collective_compute("AllReduce", OP.add, RG,
                                     ins=[d_r1i[2][:]], outs=[d_r1o[2][:]])

        # z recompute at my positions (independent of scan) — emitted here
        # so it fills engine gaps during the r1 AllReduce stall.
        t_z = {}
        zq = ctx.enter_context(tc.tile_pool(name="zq", bufs=1))
        with tc.tile_pool(name="zp", bufs=2, space="PSUM") as zpp:
            for zi, (mod, ck) in enumerate(
                    (("vi", 0), ("vi", 1), ("ir", 0), ("ir", 1))):
                xt = t_xvc if mod == "vi" else t_xic
                p_z = zpp.tile([DM, PC], F32, tag="pz2")
                nc.tensor.matmul(p_z[:], t_wz[:, zi, :], xt[:],
                                 start=True, stop=True)
                t_e = zq.tile([DM, PC], F32, tag=f"ze{zi}", name=f"ze{zi}")
                nc.scalar.activation(t_e[:], p_z[:], AF.Exp, bias=0.0, scale=-1.0)
                nc.vector.tensor_scalar_add(t_e[:], t_e[:], 1.0)
                t_r = zq.tile([DM, PC], F32, tag=f"zrr{zi}", name=f"zrr{zi}")
                nc.vector.reciprocal(t_r[:], t_e[:])
                tz = zq.tile([DM, PC], F32, tag=f"z{zi}", name=f"z{zi}")
                nc.vector.tensor_mul(tz[:], p_z[:], t_r[:])
                t_z[(mod, ck)] = tz

        # =========== PHASE B: scan middle ===========
        t_v1o = big.tile([12, 4], F32, tag="v1o")
        nc.sync.dma_start(out=t_v1o, in_=d_r1o[2][0:12, 384:388])

        def load_r1(dst, row0, row1, sp0, n, doff):
            """DMA r1 rows [row0,row1) spatial cols [sp0,sp0+n) into
            dst[:, doff:doff+n], splitting at slice boundaries."""
            off = 0
            while off < n:
                g = sp0 + off
                j = min(g // SLC, 2)
                end = SLC * (j + 1) if j < 2 else HW
                span = min(n - off, end - g)
                nc.sync.dma_start(
                    out=dst[:, doff + off:doff + off + span],
                    in_=d_r1o[j][row0:row1, g - j * SLC:g - j * SLC + span]
                    .bitcast(R32))
                off += span

        xs_t = {"sub": t_xs96, "vi": t_xv24, "ir": t_xi24}

        def xs_view(t, col, n):
            half = 1 if col >= HW else 0
            mod = (("sub", "vi"), ("sub", "ir"), ("vi", "ir"))[t][half]
            sc = col - HW * half
            return xs_t[mod][0:DSL, sc:sc + n]

        with tc.tile_pool(name="pb", bufs=3) as pb, \
             tc.tile_pool(name="pbd", bufs=2, space="PSUM") as pbd, \
             tc.tile_pool(name="pbp", bufs=1, space="PSUM") as pbp, \
             tc.tile_pool(name="pby", bufs=2, space="PSUM") as pby:
            for t in range(3):
                chunk_order = range(NCH) if t < 2 else range(NCH - 1, -1, -1)
                carry = None
                for c in chunk_order:
                    c0 = c * CH
                    # segment pieces within chunk: (start, end, k) in tile cols
                    k_lo = t if t < 2 else 2
                    k_hi = t if t < 2 else 3
                    if c0 >= HW:
                        pieces = [(c0, c0 + CH, k_hi)]
                    elif c0 + CH <= HW:
                        pieces = [(c0, c0 + CH, k_lo)]
                    else:
                        pieces = [(c0, HW, k_lo), (HW, c0 + CH, k_hi)]

                    t_rR = pb.tile([RNK, CH], R32, tag="rR")
                    t_rB = pb.tile([NST, CH], R32, tag="rB")
                    for (s, e, k) in pieces:
                        half = 1 if s >= HW else 0
                        rb = ROWBASE[(t, half)]
                        sp0 = s - HW * half
                        load_r1(t_rR, rb, rb + RNK, sp0, e - s, s - c0)
                        load_r1(t_rB, rb + RNK, rb + RNK + NST, sp0, e - s,
                                s - c0)
                    p_dts = pbd.tile([LANES, CH], F32, tag="dts")
                    for (s, e, k) in pieces:
                        nc.tensor.matmul(p_dts[:, s - c0:e - c0], t_wdtr[:, k, :],
                                         t_rR[:, s - c0:e - c0],
                                         start=True, stop=True)
                    t_et = pb.tile([LANES, CH], F32, tag="et")
                    for (s, e, k) in pieces:
                        nc.scalar.activation(t_et[:, s - c0:e - c0],
                                             p_dts[:, s - c0:e - c0], AF.Exp,
                                             bias=t_dtb[:, k:k + 1], scale=1.0)
                    t_delta = pb.tile([LANES, CH], F32, tag="delta")
                    nc.scalar.activation(t_delta[:], t_et[:], AF.Ln,
                                         bias=1.0, scale=1.0)
                    t_u = pb.tile([DSL, CH], R32, tag="u")
                    for (s, e, _k) in pieces:
                        nc.vector.tensor_mul(t_u[:, s - c0:e - c0],
                                             t_delta[0:DSL, s - c0:e - c0],
                                             xs_view(t, s, e - s).bitcast(F32))
                    p_u = pbp.tile([LANES, CH], F32, tag="urep")
                    nc.tensor.matmul(p_u[:], t_rep24[:], t_u[:], start=True, stop=True)
                    p_B = pbp.tile([LANES, CH], F32, tag="brep")
                    nc.tensor.matmul(p_B[:], t_repb[:], t_rB[:],
                                     start=True, stop=True)
                    t_bsb = pb.tile([LANES, CH], F32, tag="bsb")
                    nc.scalar.copy(t_bsb[:], p_B[:])
                    t_b = pb.tile([LANES, CH], F32, tag="b")
                    nc.vector.tensor_mul(t_b[:], p_u[:], t_bsb[:])
                    t_a = pb.tile([LANES, CH], F32, tag="a")
                    for (s, e, k) in pieces:
                        nc.scalar.activation(t_a[:, s - c0:e - c0],
                                             t_delta[:, s - c0:e - c0], AF.Exp,
                                             bias=0.0, scale=t_asc[:, k:k + 1])
                    t_h = pb.tile([LANES, CH], F32, tag="h")
                    if t < 2:
                        init = 0.0 if c == 0 else carry[:, CH - 1:CH]
                        nc.vector.tensor_tensor_scan(t_h[:], t_a[:], t_b[:], init,
                                                     OP.mult, OP.add)
                        carry = t_h
                    else:
                        # reverse scan; pieces processed right-to-left
                        for (s, e, k) in reversed(pieces):
                            sl = slice(s - c0, e - c0)
                            if e == L or e == HW:      # scan-time segment start
                                init = 0.0
                            else:
                                init = carry
                            nc.vector.tensor_tensor_scan(
                                t_h[:, sl][:, ::-1], t_a[:, sl][:, ::-1],
                                t_b[:, sl][:, ::-1], init, OP.mult, OP.add)
                            carry = t_h[:, s - c0:s - c0 + 1]

                    # y: only vi/ir halves feed the output
                    ypieces = [((s if t == 2 else max(s, HW)), e, k)
                               for (s, e, k) in pieces if t == 2 or e > HW]
                    if not ypieces:
                        continue
                    y0 = ypieces[0][0] - c0
                    y1 = ypieces[-1][1] - c0
                    t_rC = pb.tile([NST, CH], R32, tag="rC")
                    for (s, e, k) in ypieces:
                        half = 1 if s >= HW else 0
                        rb = ROWBASE[(t, half)]
                        load_r1(t_rC, rb + RNK + NST, rb + 14, s - HW * half,
                                e - s, s - c0)
                    p_C = pbp.tile([LANES, CH], F32, tag="crep")
                    nc.tensor.matmul(p_C[:, y0:y1], t_repb[:],
                                     t_rC[:, y0:y1], start=True, stop=True)
                    t_hc = pb.tile([LANES, CH], R32, tag="hc")
                    nc.vector.tensor_mul(t_hc[:, y0:y1], t_h[:, y0:y1],
                                         p_C[:, y0:y1])
                    p_y = pby.tile([DSL, CH], F32, tag="y")
                    nc.tensor.matmul(p_y[:, y0:y1], t_m96[:], t_hc[:, y0:y1],
                                     start=True, stop=True)
                    # evacuate/accumulate into y_vi / y_ir; on fwd tiles the
                    # D-skip (combined D_k + D_{k+2}) folds into the same op:
                    # y = xs * D + p_y
                    for (s, e, _k) in ypieces:
                        sl = slice(s - c0, e - c0)
                        if t < 2:
                            yt = t_yvi if t == 0 else t_yir
                            nc.vector.scalar_tensor_tensor(
                                yt[:, s - HW:e - HW],
                                xs_view(t, s, e - s).bitcast(F32),
                                t_dvec[:, t:t + 1], p_y[:, sl],
                                OP.mult, OP.add)
                        elif s < HW:  # t2 k2 -> vi
                            nc.vector.tensor_add(t_yvi[:, s:e], t_yvi[:, s:e],
                                                 p_y[:, sl])
                        else:         # t2 k3 -> ir
                            nc.vector.tensor_add(t_yir[:, s - HW:e - HW],
                                                 t_yir[:, s - HW:e - HW], p_y[:, sl])

        # =========== A2A: reshard y channels -> positions ===========
        for j in range(NCORES):
            nc.sync.dma_start(out=d_a2i[j, 0:DSL, :],
                              in_=t_yvi[:, j * PC:(j + 1) * PC])
            nc.sync.dma_start(out=d_a2i[j, DSL:2 * DSL, :],
                              in_=t_yir[:, j * PC:(j + 1) * PC])
        nc.gpsimd.collective_compute("AllToAll", OP.bypass, RG,
                                     ins=[d_a2i[:]], outs=[d_a2o[:]])

        # =========== PHASE C: LN + gate + out (position-sharded) ===========
        with tc.tile_pool(name="pcq", bufs=2) as pcq, \
             tc.tile_pool(name="pcp", bufs=1, space="PSUM") as pcp:
            # gather y chunks [96, PC] x (2 chunks, 2 mods)
            t_y = {}
            for mod, roff in (("vi", 0), ("ir", DSL)):
                for ck in range(2):
                    ty = pcq.tile([DM, PC], R32, tag=f"y{mod}{ck}", name=f"y{mod}{ck}")
                    for jj in range(4):
                        j = ck * 4 + jj
                        nc.sync.dma_start(out=ty[jj * DSL:(jj + 1) * DSL, :],
                                          in_=d_a2o[j, roff:roff + DSL, :]
                                          .bitcast(R32))
                    t_y[(mod, ck)] = ty
            # chan-attn scales s = 1 + sigmoid(f2 @ (relu(va)+relu(vm)))
            t_vr = pcq.tile([12, 4], F32, tag="vr")
            nc.scalar.activation(t_vr[:], t_v1o[:], AF.Relu)
            t_vw = pcq.tile([12, 2], F32, tag="vw")
            nc.vector.tensor_add(t_vw[:, 0:1], t_vr[:, 0:1], t_vr[:, 1:2])
            nc.vector.tensor_add(t_vw[:, 1:2], t_vr[:, 2:3], t_vr[:, 3:4])
            t_s = {}
            for ck in range(2):
                p_ca = pcp.tile([DM, 2], F32, tag="pca")
                for mod_i in range(2):
                    nc.tensor.matmul(p_ca[:, mod_i:mod_i + 1], t_f2[:, mod_i, ck, :],
                                     t_vw[:, mod_i:mod_i + 1],
                                     start=True, stop=True)
                t_e = pcq.tile([DM, 2], F32, tag="cae")
                nc.scalar.activation(t_e[:], p_ca[:], AF.Exp, bias=0.0, scale=-1.0)
                nc.vector.tensor_scalar_add(t_e[:], t_e[:], 1.0)
                t_r = pcq.tile([DM, 2], F32, tag=f"car{ck}", name=f"car{ck}")
                nc.vector.reciprocal(t_r[:], t_e[:])          # sigmoid
                nc.vector.tensor_scalar_add(t_r[:], t_r[:], 1.0)  # 1 + sigmoid
                t_s[ck] = t_r
            # LN per modality
            t_fin = {}
            for mod in ("vi", "ir"):
                p_s1 = pcp.tile([1, PC], F32, tag="s1")
                p_s2 = pcp.tile([1, PC], F32, tag="s2")
                for ck in range(2):
                    nc.tensor.matmul(p_s1[:], t_onec[:],
                                     t_y[(mod, ck)][:], start=(ck == 0),
                                     stop=(ck == 1))
                for ck in range(2):
                    t_sq = pcq.tile([DM, PC], R32, tag="sq")
                    nc.scalar.activation(t_sq[:], t_y[(mod, ck)][:].bitcast(F32),
                                         AF.Square)
                    nc.tensor.matmul(p_s2[:], t_onec[:],
                                     t_sq[:], start=(ck == 0), stop=(ck == 1))
                t_mu = pcq.tile([1, PC], F32, tag="mu")
                nc.vector.tensor_scalar_mul(t_mu[:], p_s1[:], 1.0 / DI)
                t_musq = pcq.tile([1, PC], F32, tag="musq")
                nc.vector.tensor_mul(t_musq[:], t_mu[:], t_mu[:])
                t_var = pcq.tile([1, PC], F32, tag="var")
                nc.vector.scalar_tensor_tensor(t_var[:], p_s2[:], 1.0 / DI,
                                               t_musq[:], OP.mult, OP.subtract)
                t_eps = pcq.tile([1, 1], F32, tag="eps")
                nc.vector.memset(t_eps[:], 1e-5)
                t_lnv = pcq.tile([1, PC], F32, tag="lnv")
                nc.scalar.activation(t_lnv[:], t_var[:], AF.Ln, bias=t_eps[:], scale=1.0)
                t_rstd = pcq.tile([1, PC], R32, tag="rstd")
                nc.scalar.activation(t_rstd[:], t_lnv[:], AF.Exp, bias=0.0, scale=-0.5)
                t_mur = pcq.tile([1, PC], R32, tag="mur")
                nc.vector.tensor_mul(t_mur[:], t_mu[:], t_rstd[:].bitcast(F32))
                p_q = pcp.tile([DM, PC], F32, tag="pq")
                nc.tensor.matmul(p_q[:], t_oner[:], t_rstd[:], start=True, stop=True)
                p_m = pcp.tile([DM, PC], F32, tag="pm")
                nc.tensor.matmul(p_m[:], t_oner[:], t_mur[:], start=True, stop=True)
                gb = {"vi": (0, 1), "ir": (2, 3)}[mod]
                for ck in range(2):
                    t_t = pcq.tile([DM, PC], F32, tag="lt")
                    nc.vector.tensor_mul(t_t[:], t_y[(mod, ck)][:].bitcast(F32), p_q[:])
                    t_t2 = pcq.tile([DM, PC], F32, tag="lt2")
                    nc.vector.tensor_sub(t_t2[:], t_t[:], p_m[:])
                    t_yn = pcq.tile([DM, PC], F32, tag="yn")
                    nc.scalar.activation(t_yn[:], t_t2[:], AF.Identity,
                                         bias=t_lnw[:, ck, gb[1]:gb[1] + 1],
                                         scale=t_lnw[:, ck, gb[0]:gb[0] + 1])
                    # gate: fin += yn * z * s
                    t_m1 = pcq.tile([DM, PC], F32, tag="m1")
                    nc.vector.tensor_mul(t_m1[:], t_yn[:], t_z[(mod, ck)][:])
                    if mod == "vi":
                        t_f = pcq.tile([DM, PC], F32, tag=f"fin{ck}", name=f"fin{ck}")
                        nc.vector.tensor_scalar_mul(t_f[:], t_m1[:],
                                                    t_s[ck][:, 0:1])
                        t_fin[ck] = t_f
                    else:
                        nc.vector.scalar_tensor_tensor(t_fin[ck][:], t_m1[:],
                                                       t_s[ck][:, 1:2], t_fin[ck][:],
                                                       OP.mult, OP.add)
            p_o = pcp.tile([DM, PC], F32, tag="po")
            for ck in range(2):
                t_finr = pcq.tile([DM, PC], R32, tag=f"finr{ck}", name=f"finr{ck}")
                nc.vector.tensor_copy(t_finr[:], t_fin[ck][:])
                nc.tensor.matmul(p_o[:], t_wout[:, ck, :], t_finr[:],
                                 start=(ck == 0), stop=(ck == 1))
            t_o = pcq.tile([DM, PC], F32, tag="o")
            nc.scalar.copy(t_o[:], p_o[:])
            nc.sync.dma_start(out=o_out[:], in_=t_o[:])

    nc.finalize()
    return nc


def _prep_inputs(inputs):
    """Host-side prep: slice/transpose weights per core. Returns in_maps."""
    g = {k: np.asarray(v, dtype=np.float32) for k, v in inputs.items()}
    x_vi = g["x_vi"].reshape(HW, DM)
    x_ir = g["x_ir"].reshape(HW, DM)
    xvt = np.ascontiguousarray(x_vi.T)
    xit = np.ascontiguousarray(x_ir.T)
    A = -np.exp(g["A_logs"]).reshape(K, DI, NST)
    Ds = g["Ds"].reshape(K, DI)
    in_maps = []
    for c in range(NCORES):
        S = slice(c * DSL, (c + 1) * DSL)
        m = {}
        m["xvt"], m["xit"] = xvt, xit
        # packed in_proj lhsT: x blocks sub@0 vi@32 ir@64, z cols [zv | zi]
        wxA = np.zeros((DM, 96), np.float32)
        wxB = np.zeros((DM, 96), np.float32)
        wzA = np.zeros((DM, 48), np.float32)
        wzB = np.zeros((DM, 48), np.float32)
        wxA[:, 0:24] = g["W_sub"][S].T
        wxA[:, 32:56] = g["W_vi"][S].T
        wxB[:, 0:24] = -g["W_sub"][S].T
        wxB[:, 64:88] = g["W_ir"][S].T
        wzA[:, 0:24] = g["W_vi"][DI:][S].T
        wzB[:, 24:48] = g["W_ir"][DI:][S].T
        m["wxA"], m["wxB"], m["wzA"], m["wzB"] = wxA, wxB, wzA, wzB
        w9p = np.zeros((96, 9, 96), np.float32)
        b72p = np.zeros((96, 1), np.float32)
        for nm in ("sub", "vi", "ir"):
            mo = MODOFF[nm]
            cw = g[f"conv_w_{nm}"][S, 0]      # [DSL, 3, 3]
            for tap in range(9):
                for d in range(DSL):
                    w9p[mo + d, tap, mo + d] = cw[d, tap // 3, tap % 3]
            b72p[mo:mo + DSL, 0] = g[f"conv_b_{nm}"][S]
        m["w9p"], m["b72p"] = w9p, b72p
        # x_dbl block lhsT: 6 groups (k0s k0v k1s k1i k2v k3i) x 14 rows
        wpk84 = np.zeros((96, GR), np.float32)
        for gi2, (k, nm) in enumerate(
                ((0, "sub"), (0, "vi"), (1, "sub"), (1, "ir"),
                 (2, "vi"), (3, "ir"))):
            mo = MODOFF[nm]
            wpk84[mo:mo + DSL, gi2 * 14:(gi2 + 1) * 14] = \
                g["x_proj_weight"][k][:, S].T
        m["wpk84"] = wpk84
        wdtr = np.zeros((RNK, K, LANES), np.float32)
        dtb = np.zeros((LANES, K), np.float32)
        asc = np.zeros((LANES, K), np.float32)
        for k in range(K):
            for n in range(NST):
                for d in range(DSL):
                    lane = n * DSL + d
                    wdtr[:, k, lane] = g["dt_projs_weight"][k, c * DSL + d, :]
                    dtb[lane, k] = g["dt_projs_bias"][k, c * DSL + d]
                    asc[lane, k] = A[k, c * DSL + d, n]
        m["wdtr"], m["dtb"], m["asc"] = wdtr, dtb, asc
        rep24 = np.zeros((DSL, LANES), np.float32)
        repb = np.zeros((NST, LANES), np.float32)
        m96 = np.zeros((LANES, DSL), np.float32)
        for n in range(NST):
            for d in range(DSL):
                rep24[d, n * DSL + d] = 1
                repb[n, n * DSL + d] = 1
                m96[n * DSL + d, d] = 1
        m["rep24"], m["repb"], m["m96"] = rep24, repb, m96
        dvec = np.zeros((DSL, 2), np.float32)
        dvec[:, 0] = Ds[0, S] + Ds[2, S]
        dvec[:, 1] = Ds[1, S] + Ds[3, S]
        m["dvec"] = dvec
        f1q = np.zeros((48, 2, 12), np.float32)
        f1q[0:24, 0] = g["ca_vi_f1"][:, S].T
        f1q[24:48, 1] = g["ca_ir_f1"][:, S].T
        m["f1q"] = f1q
        f2 = np.zeros((12, 2, 2, DM), np.float32)
        for ck in range(2):
            f2[:, 0, ck] = g["ca_vi_f2"][ck * DM:(ck + 1) * DM].T
            f2[:, 1, ck] = g["ca_ir_f2"][ck * DM:(ck + 1) * DM].T
        m["f2"] = f2
        lnw = np.zeros((DM, 2, 4), np.float32)
        for ck in range(2):
            cs = slice(ck * DM, (ck + 1) * DM)
            lnw[:, ck, 0] = g["ln_vi_g"][cs]
            lnw[:, ck, 1] = g["ln_vi_b"][cs]
            lnw[:, ck, 2] = g["ln_ir_g"][cs]
            lnw[:, ck, 3] = g["ln_ir_b"][cs]
        m["lnw"] = lnw
        wout = np.zeros((DM, 2, DM), np.float32)
        for ck in range(2):
            wout[:, ck] = g["W_out"][:, ck * DM:(ck + 1) * DM].T
        m["wout"] = wout
        wz = np.zeros((DM, 4, DM), np.float32)
        wz[:, 0] = g["W_vi"][DI:][0:DM].T
        wz[:, 1] = g["W_vi"][DI:][DM:DI].T
        wz[:, 2] = g["W_ir"][DI:][0:DM].T
        wz[:, 3] = g["W_ir"][DI:][DM:DI].T
        m["wz"] = wz
        m["onec"] = np.ones((DM, 1), np.float32)
        m["oner"] = np.ones((1, DM), np.float32)
        m["xvc"] = np.ascontiguousarray(xvt[:, c * PC:(c + 1) * PC])
        m["xic"] = np.ascontiguousarray(xit[:, c * PC:(c + 1) * PC])
        in_maps.append(m)
    return in_maps


def kernel(**inputs):
    if "nc" not in _cache:
        _cache["nc"] = _build()
    nc = _cache["nc"]
    in_maps = _prep_inputs(inputs)
    res = run_bass_kernel_spmd(nc, in_maps, core_ids=list(range(NCORES)))
    out = np.zeros((DM, HW), np.float32)
    for c in range(NCORES):
        out[:, c * PC:(c + 1) * PC] = res.results[c]["out"]
    return out.T.reshape(B, H, W, DM).astype(np.float32)


# revision 11
# speedup vs baseline: 2.8620x; 1.1049x over previous
"""Trainium2 Bass kernel for the DSSM (dual-modality Mamba-style 2D selective
scan) module. 8-core SPMD: scan channels d-sharded (24/core x 4 directions),
upstream in_proj/dwconv d-sharded with modalities packed into partitions,
downstream LN/out position-sharded. Cross-core: 3 column-sliced AllReduces of
x_dbl partials (pipelined against phase A) and one AllToAll (y reshard).
All wide matmuls run as float32r (full-rate fp32 on the PE array).
"""
import sys
sys.path.insert(0, "/opt/trn_rl_repo")
import numpy as np
import concourse.bass as bass
from concourse import mybir
from concourse.bacc import Bacc
from concourse.tile import TileContext
from concourse.bass_utils import run_bass_kernel_spmd

F32 = mybir.dt.float32
R32 = mybir.dt.float32r
F16 = mybir.dt.float16
AF = mybir.ActivationFunctionType
OP = mybir.AluOpType

NCORES = 8
RG = [list(range(NCORES))]
B, H, W = 1, 48, 48
HW = H * W                      # 2304
L = 2 * HW                      # 4608
DM = 96                         # d_model
DI = 192                        # d_inner
NST = 4                         # d_state
RNK = 6                         # dt_rank
K = 4
DSL = DI // NCORES              # 24 channels per core
LANES = NST * DSL               # 96 scan lanes (lane = n*DSL + d)
CH = 512                        # phase-B column chunk
NCH = L // CH                   # 9
PC = HW // NCORES               # 288 positions per core (phase C)
RCH = 480                       # phase-A chunk = 10 image rows
ROWCHUNKS = [(0, 10), (10, 10), (20, 10), (30, 10), (40, 8)]
# r1 layout: [84 rows, spatial cols] split in 3 col slices (pipelined AR).
# rows: 6 groups x 14 (dts 0:6 | B 6:10 | C 10:14), group order:
# (k0,sub) (k0,vi) (k1,sub) (k1,ir) (k2,vi) (k3,ir)
GR = 84
SLC = 960                       # r1 col-slice width (2 rowchunks)
SL_COLS = [960, 960, 388]       # slice2: 384 spatial + 4 attn cols
ROWBASE = {(0, 0): 0, (0, 1): 14, (1, 0): 28, (1, 1): 42,
           (2, 0): 56, (2, 1): 70}  # (tile, half) -> r1 row group base
MODOFF = {"sub": 0, "vi": 32, "ir": 64}  # 32-aligned partition blocks

_cache = {}


def _patch_act_tables():
    import concourse.bacc as _bacc
    from concourse.hw_specs import get_activation_tables as _gat
    if getattr(_bacc, "_act_tables_patched", False):
        return
    def patched(arch):
        tabs = {k: set(v) for k, v in _gat(arch).items()}
        # Force exp/ln to resolve to the combined natural_log_exp table so
        # softplus chains (exp -> ln -> exp) never reload act tables.
        for name in ("exp_and_others", "exp_and_friends"):
            if name in tabs:
                tabs[name].discard(AF.Exp)
        if "natural_log" in tabs:
            tabs["natural_log"].discard(AF.Ln)
        return tabs
    _bacc.get_activation_tables = patched
    _bacc._act_tables_patched = True


def _build():
    _patch_act_tables()
    nc = Bacc(trn_type="TRN2", num_devices=NCORES)
    EIn = dict(kind="ExternalInput")
    # per-core inputs (host-prepped)
    i_xvt = nc.dram_tensor("xvt", [DM, HW], F16, **EIn)
    i_xit = nc.dram_tensor("xit", [DM, HW], F16, **EIn)
    i_wxA = nc.dram_tensor("wxA", [DM, 96], F16, **EIn)  # x in_proj vs xvt
    i_wxB = nc.dram_tensor("wxB", [DM, 96], F16, **EIn)  # x in_proj vs xit
    i_wzA = nc.dram_tensor("wzA", [DM, 48], F16, **EIn)  # z in_proj vs xvt
    i_wzB = nc.dram_tensor("wzB", [DM, 48], F16, **EIn)  # z in_proj vs xit
    i_w9p = nc.dram_tensor("w9p", [96, 9, 96], F32, **EIn)  # conv diag, mods packed
    i_b72p = nc.dram_tensor("b72p", [96, 1], F32, **EIn)
    i_wpk84 = nc.dram_tensor("wpk84", [96, GR], F32, **EIn)  # x_dbl block lhsT
    i_wdtr = nc.dram_tensor("wdtr", [RNK, K, LANES], F16, **EIn)
    i_dtb = nc.dram_tensor("dtb", [LANES, K], F32, **EIn)
    i_asc = nc.dram_tensor("asc", [LANES, K], F32, **EIn)
    i_rep24 = nc.dram_tensor("rep24", [DSL, LANES], F16, **EIn)
    i_repb = nc.dram_tensor("repb", [NST, LANES], F16, **EIn)
    i_m96 = nc.dram_tensor("m96", [LANES, DSL], F16, **EIn)
    i_dvec = nc.dram_tensor("dvec", [DSL, 2], F32, **EIn)  # (vi,ir) summed D
    i_f1q = nc.dram_tensor("f1q", [48, 2, 12], F32, **EIn)  # attn mlp1 per mod
    i_f2 = nc.dram_tensor("f2", [12, 2, 2, DM], F32, **EIn)  # (mod, chunk, out96)
    i_lnw = nc.dram_tensor("lnw", [DM, 2, 4], F32, **EIn)    # (chunk, gvi bvi gir bir)
    i_wout = nc.dram_tensor("wout", [DM, 2, DM], F32, **EIn)  # (contract chunk, out)
    i_wz = nc.dram_tensor("wz", [DM, 4, DM], F16, **EIn)     # z lhsT (vi0,vi1,ir0,ir1)
    i_onec = nc.dram_tensor("onec", [DM, 1], F16, **EIn)
    i_oner = nc.dram_tensor("oner", [1, DM], F32, **EIn)
    i_xvc = nc.dram_tensor("xvc", [DM, PC], F16, **EIn)
    i_xic = nc.dram_tensor("xic", [DM, PC], F16, **EIn)
    o_out = nc.dram_tensor("out", [DM, PC], F32, kind="ExternalOutput")
    # collective DRAM buffers (one pair per r1 col-slice)
    d_r1i = [nc.dram_tensor(f"d_r1i{j}", [GR, SL_COLS[j]], F16)
             for j in range(3)]
    d_r1o = [nc.dram_tensor(f"d_r1o{j}", [GR, SL_COLS[j]], F16,
                            addr_space="Shared") for j in range(3)]
    d_a2i = nc.dram_tensor("d_a2i", [NCORES, 2 * DSL, PC], F16)
    d_a2o = nc.dram_tensor("d_a2o", [NCORES, 2 * DSL, PC], F16)

    import contextlib
    with TileContext(nc) as tc, contextlib.ExitStack() as ctx:
        wpool = ctx.enter_context(tc.tile_pool(name="weights", bufs=1))
        big = ctx.enter_context(tc.tile_pool(name="big", bufs=1))

        # ---- load weights ----
        def wtile(shape, src, dt=F32):
            t = wpool.tile(shape, dt, tag=src.name, name="w_" + src.name)
            nc.sync.dma_start(out=t, in_=src[:].bitcast(dt) if dt is R32
                              else src[:])
            return t
        t_wxA = wtile([DM, 96], i_wxA, F16)
        t_wxB = wtile([DM, 96], i_wxB, F16)
        t_wzA = wtile([DM, 48], i_wzA, F16)
        t_wzB = wtile([DM, 48], i_wzB, F16)
        t_w9p = wtile([96, 9, 96], i_w9p, R32)
        t_b72p = wtile([96, 1], i_b72p)
        t_wpk84 = wtile([96, GR], i_wpk84, R32)
        t_wdtr = wtile([RNK, K, LANES], i_wdtr, F16)
        t_dtb = wtile([LANES, K], i_dtb)
        t_asc = wtile([LANES, K], i_asc)
        t_rep24 = wtile([DSL, LANES], i_rep24, F16)
        t_repb = wtile([NST, LANES], i_repb, F16)
        t_m96 = wtile([LANES, DSL], i_m96, F16)
        t_dvec = wtile([DSL, 2], i_dvec)
        t_f1q = wtile([48, 2, 12], i_f1q)
        t_f2 = wtile([12, 2, 2, DM], i_f2)
        t_lnw = wtile([DM, 2, 4], i_lnw)
        t_wout = wtile([DM, 2, DM], i_wout, R32)
        t_wz = wtile([DM, 4, DM], i_wz, F16)
        t_onec = wtile([DM, 1], i_onec, F16)
        t_oner = wtile([1, DM], i_oner, R32)
        t_xvc = wtile([DM, PC], i_xvc, F16)
        t_xic = wtile([DM, PC], i_xic, F16)

        # sliced input loads (per rowchunk) so phase A starts early
        t_xvt = big.tile([DM, HW], F16)
        t_xit = big.tile([DM, HW], F16)
        for (r0, nr) in ROWCHUNKS:
            cs = slice(r0 * W, (r0 + nr) * W)
            nc.sync.dma_start(out=t_xvt[:, cs], in_=i_xvt[:, cs])
            nc.sync.dma_start(out=t_xit[:, cs], in_=i_xit[:, cs])

        # persistent SBUF
        t_xs96 = big.tile([96, HW], R32, tag="xs96")  # rows: sub@0|vi@32|ir@64
        t_xv24 = big.tile([DSL, HW], R32, tag="xv24")  # base-0 copy of vi block
        t_xi24 = big.tile([DSL, HW], R32, tag="xi24")  # base-0 copy of ir block
        t_yvi = big.tile([DSL, HW], F16, tag="yvi")
        t_yir = big.tile([DSL, HW], F16, tag="yir")

        # =========== PHASE A: upstream (d-sharded, mods packed) ===========
        pa1 = ctx.enter_context(tc.tile_pool(name="pa1", bufs=1))
        with tc.tile_pool(name="pain", bufs=2, space="PSUM") as pain, \
             tc.tile_pool(name="painz", bufs=2, space="PSUM") as painz, \
             tc.tile_pool(name="pacv", bufs=2, space="PSUM") as pacv, \
             tc.tile_pool(name="padb", bufs=2, space="PSUM") as padb:
            t_pad = pa1.tile([96, 50, 50], R32, tag="pad")
            nc.vector.memset(t_pad[:].bitcast(F32), 0.0)

            t_zc = pa1.tile([48, HW], F32, tag="zc")  # silu(z), vi|ir packed
            t_zacc = pa1.tile([48, len(ROWCHUNKS)], F32, tag="zacc")
            # in_proj (+z) packed: x=[sub|vi|ir], z=[zv|zi]
            for ic, (r0, nr) in enumerate(ROWCHUNKS):
                cols = slice(r0 * W, (r0 + nr) * W)
                p_x = pain.tile([96, RCH], F32, tag="pin")
                nc.tensor.matmul(p_x[:, :nr * W], t_wxA[:], t_xvt[:, cols],
                                 start=True, stop=False)
                nc.tensor.matmul(p_x[:, :nr * W], t_wxB[:], t_xit[:, cols],
                                 start=False, stop=True)
                p_z = painz.tile([48, RCH], F32, tag="pz")
                nc.tensor.matmul(p_z[:, :nr * W], t_wzA[:], t_xvt[:, cols],
                                 start=True, stop=False)
                nc.tensor.matmul(p_z[:, :nr * W], t_wzB[:], t_xit[:, cols],
                                 start=False, stop=True)
                nc.scalar.activation(t_zc[:, cols], p_z[:, :nr * W],
                                     AF.Silu, accum_out=t_zacc[:, ic:ic + 1])
                nc.scalar.copy(
                    t_pad[:, 1 + r0:1 + r0 + nr, 1:49],
                    p_x[:, :nr * W].rearrange("p (a b) -> p a b", a=nr))

            # depthwise conv 3x3 (9 block-diag matmuls) + bias + silu -> xs
            for (r0, nr) in ROWCHUNKS:
                p_c = pacv.tile([96, RCH], F32, tag="pconv")
                for tap in range(9):
                    dy, dx = tap // 3, tap % 3
                    nc.tensor.matmul(
                        p_c[:, :nr * W], t_w9p[:, tap, :],
                        t_pad[:, r0 + dy:r0 + dy + nr, dx:dx + 48],
                        start=(tap == 0), stop=(tap == 8))
                cols = slice(r0 * W, (r0 + nr) * W)
                nc.scalar.activation(
                    t_xs96[:, cols], p_c[:, :nr * W],
                    AF.Silu, bias=t_b72p[:], scale=1.0)
                nc.vector.tensor_copy(t_xv24[:, cols], t_xs96[32:56, cols])
                nc.vector.tensor_copy(t_xi24[:, cols], t_xs96[64:88, cols])

            # x_dbl all 6 groups in one matmul per rowchunk -> r1 slices
            for ic, (r0, nr) in enumerate(ROWCHUNKS):
                p_d = padb.tile([GR, RCH], F32, tag="pxdbl")
                nc.tensor.matmul(p_d[:, :nr * W], t_wpk84[:],
                                 t_xs96[:, r0 * W:(r0 + nr) * W],
                                 start=True, stop=True)
                t_xe = pa1.tile([GR, RCH], F16, tag=f"txe{ic}", name=f"txe{ic}")
                nc.scalar.copy(t_xe[:, :nr * W], p_d[:, :nr * W])
                j = (r0 * W) // SLC
                lc = r0 * W - j * SLC
                nc.sync.dma_start(out=d_r1i[j][:, lc:lc + nr * W],
                                  in_=t_xe[:, :nr * W])
                if ic == 1:
                    nc.gpsimd.collective_compute(
                        "AllReduce", OP.add, RG,
                        ins=[d_r1i[0][:]], outs=[d_r1o[0][:]])
                elif ic == 3:
                    nc.gpsimd.collective_compute(
                        "AllReduce", OP.add, RG,
                        ins=[d_r1i[1][:]], outs=[d_r1o[1][:]])
        # chan-attn pooled stats + v1 partials [12, 4] (own PSUM scope)
        with tc.tile_pool(name="pav1", bufs=1, space="PSUM") as pav1:
            t_pool = pa1.tile([48, 2], F32, tag="tpool")  # (avg, max)
            nc.vector.tensor_reduce(t_pool[:, 0:1], t_zacc[:],
                                    axis=mybir.AxisListType.X, op=OP.add)
            nc.vector.tensor_scalar_mul(t_pool[:, 0:1], t_pool[:, 0:1], 1.0 / HW)
            nc.vector.tensor_reduce(t_pool[:, 1:2], t_zc[:],
                                    axis=mybir.AxisListType.X, op=OP.max)
            t_v1 = pa1.tile([12, 4], F16, tag="tv1")  # (via, vim, ira, irm)
            p_v1 = pav1.tile([12, 4], F32, tag="pv1")
            for mi in range(2):
                for st in range(2):
                    nc.tensor.matmul(p_v1[:, 2 * mi + st:2 * mi + st + 1],
                                     t_f1q[:, mi, :], t_pool[:, st:st + 1],
                                     start=True, stop=True)
            nc.scalar.copy(t_v1[:], p_v1[:])
            nc.sync.dma_start(out=d_r1i[2][0:12, 384:388], in_=t_v1[:])
        nc.gpsimd.collective_compute("AllReduce", OP.add, RG,
                                     ins=[d_r1i[2][:]], outs=[d_r1o[2][:]])

        # z recompute at my positions (independent of scan) — emitted here
        # so it fills engine gaps during the r1 AllReduce stall.
        t_z = {}
        zq = ctx.enter_context(tc.tile_pool(name="zq", bufs=1))
        with tc.tile_pool(name="zp", bufs=2, space="PSUM") as zpp:
            for zi, (mod, ck) in enumerate(
                    (("vi", 0), ("vi", 1), ("ir", 0), ("ir", 1))):
                xt = t_xvc if mod == "vi" else t_xic
                p_z = zpp.tile([DM, PC], F32, tag="pz2")
                nc.tensor.matmul(p_z[:], t_wz[:, zi, :], xt[:],
                                 start=True, stop=True)
                t_e = zq.tile([DM, PC], F32, tag=f"ze{zi}", name=f"ze{zi}")
                nc.scalar.activation(t_e[:], p_z[:], AF.Exp, bias=0.0, scale=-1.0)
                nc.vector.tensor_scalar_add(t_e[:], t_e[:], 1.0)
                t_r = zq.tile([DM, PC], F32, tag=f"zrr{zi}", name=f"zrr{zi}")
                nc.vector.reciprocal(t_r[:], t_e[:])
                tz = zq.tile([DM, PC], F32, tag=f"z{zi}", name=f"z{zi}")
                nc.vector.tensor_mul(tz[:], p_z[:], t_r[:])
                t_z[(mod, ck)] = tz

        # =========== PHASE B: scan middle ===========
        t_v1o = big.tile([12, 4], F16, tag="v1o")
        nc.sync.dma_start(out=t_v1o, in_=d_r1o[2][0:12, 384:388])

        def load_r1(dst, row0, row1, sp0, n, doff):
            """DMA r1 rows [row0,row1) spatial cols [sp0,sp0+n) into
            dst[:, doff:doff+n], splitting at slice boundaries."""
            off = 0
            while off < n:
                g = sp0 + off
                j = min(g // SLC, 2)
                end = SLC * (j + 1) if j < 2 else HW
                span = min(n - off, end - g)
                nc.sync.dma_start(
                    out=dst[:, doff + off:doff + off + span],
                    in_=d_r1o[j][row0:row1, g - j * SLC:g - j * SLC + span])
                off += span

        xs_t = {"sub": t_xs96, "vi": t_xv24, "ir": t_xi24}

        def xs_view(t, col, n):
            half = 1 if col >= HW else 0
            mod = (("sub", "vi"), ("sub", "ir"), ("vi", "ir"))[t][half]
            sc = col - HW * half
            return xs_t[mod][0:DSL, sc:sc + n]

        with tc.tile_pool(name="pb", bufs=3) as pb, \
             tc.tile_pool(name="pbd", bufs=2, space="PSUM") as pbd, \
             tc.tile_pool(name="pbp", bufs=1, space="PSUM") as pbp, \
             tc.tile_pool(name="pby", bufs=2, space="PSUM") as pby:
            for t in range(3):
                chunk_order = range(NCH) if t < 2 else range(NCH - 1, -1, -1)
                carry = None
                for c in chunk_order:
                    c0 = c * CH
                    # segment pieces within chunk: (start, end, k) in tile cols
                    k_lo = t if t < 2 else 2
                    k_hi = t if t < 2 else 3
                    if c0 >= HW:
                        pieces = [(c0, c0 + CH, k_hi)]
                    elif c0 + CH <= HW:
                        pieces = [(c0, c0 + CH, k_lo)]
                    else:
                        pieces = [(c0, HW, k_lo), (HW, c0 + CH, k_hi)]

                    t_rR = pb.tile([RNK, CH], F16, tag="rR")
                    t_rB = pb.tile([NST, CH], F16, tag="rB")
                    for (s, e, k) in pieces:
                        half = 1 if s >= HW else 0
                        rb = ROWBASE[(t, half)]
                        sp0 = s - HW * half
                        load_r1(t_rR, rb, rb + RNK, sp0, e - s, s - c0)
                        load_r1(t_rB, rb + RNK, rb + RNK + NST, sp0, e - s,
                                s - c0)
                    p_dts = pbd.tile([LANES, CH], F32, tag="dts")
                    for (s, e, k) in pieces:
                        nc.tensor.matmul(p_dts[:, s - c0:e - c0], t_wdtr[:, k, :],
                                         t_rR[:, s - c0:e - c0],
                                         start=True, stop=True)
                    t_et = pb.tile([LANES, CH], F32, tag="et")
                    for (s, e, k) in pieces:
                        nc.scalar.activation(t_et[:, s - c0:e - c0],
                                             p_dts[:, s - c0:e - c0], AF.Exp,
                                             bias=t_dtb[:, k:k + 1], scale=1.0)
                    t_delta = pb.tile([LANES, CH], F32, tag="delta")
                    nc.scalar.activation(t_delta[:], t_et[:], AF.Ln,
                                         bias=1.0, scale=1.0)
                    t_u = pb.tile([DSL, CH], F16, tag="u")
                    for (s, e, _k) in pieces:
                        nc.vector.tensor_mul(t_u[:, s - c0:e - c0],
                                             t_delta[0:DSL, s - c0:e - c0],
                                             xs_view(t, s, e - s).bitcast(F32))
                    p_u = pbp.tile([LANES, CH], F32, tag="urep")
                    nc.tensor.matmul(p_u[:], t_rep24[:], t_u[:], start=True, stop=True)
                    p_B = pbp.tile([LANES, CH], F32, tag="brep")
                    nc.tensor.matmul(p_B[:], t_repb[:], t_rB[:],
                                     start=True, stop=True)
                    t_bsb = pb.tile([LANES, CH], F16, tag="bsb")
                    nc.scalar.copy(t_bsb[:], p_B[:])
                    t_b = pb.tile([LANES, CH], F16, tag="b")
                    nc.vector.tensor_mul(t_b[:], p_u[:], t_bsb[:])
                    t_a = pb.tile([LANES, CH], F16, tag="a")
                    for (s, e, k) in pieces:
                        nc.scalar.activation(t_a[:, s - c0:e - c0],
                                             t_delta[:, s - c0:e - c0], AF.Exp,
                                             bias=0.0, scale=t_asc[:, k:k + 1])
                    t_h = pb.tile([LANES, CH], F16, tag="h")
                    if t < 2:
                        init = 0.0 if c == 0 else carry[:, CH - 1:CH]
                        nc.vector.tensor_tensor_scan(t_h[:], t_a[:], t_b[:], init,
                                                     OP.mult, OP.add)
                        carry = t_h
                    else:
                        # reverse scan; pieces processed right-to-left
                        for (s, e, k) in reversed(pieces):
                            sl = slice(s - c0, e - c0)
                            if e == L or e == HW:      # scan-time segment start
                                init = 0.0
                            else:
                                init = carry
                            nc.vector.tensor_tensor_scan(
                                t_h[:, sl][:, ::-1], t_a[:, sl][:, ::-1],
                                t_b[:, sl][:, ::-1], init, OP.mult, OP.add)
                            carry = t_h[:, s - c0:s - c0 + 1]

                    # y: only vi/ir halves feed the output
                    ypieces = [((s if t == 2 else max(s, HW)), e, k)
                               for (s, e, k) in pieces if t == 2 or e > HW]
                    if not ypieces:
                        continue
                    y0 = ypieces[0][0] - c0
                    y1 = ypieces[-1][1] - c0
                    t_rC = pb.tile([NST, CH], F16, tag="rC")
                    for (s, e, k) in ypieces:
                        half = 1 if s >= HW else 0
                        rb = ROWBASE[(t, half)]
                        load_r1(t_rC, rb + RNK + NST, rb + 14, s - HW * half,
                                e - s, s - c0)
                    p_C = pbp.tile([LANES, CH], F32, tag="crep")
                    nc.tensor.matmul(p_C[:, y0:y1], t_repb[:],
                                     t_rC[:, y0:y1], start=True, stop=True)
                    t_hc = pb.tile([LANES, CH], F16, tag="hc")
                    nc.vector.tensor_mul(t_hc[:, y0:y1], t_h[:, y0:y1],
                                         p_C[:, y0:y1])
                    p_y = pby.tile([DSL, CH], F32, tag="y")
                    nc.tensor.matmul(p_y[:, y0:y1], t_m96[:], t_hc[:, y0:y1],
                                     start=True, stop=True)
                    # evacuate/accumulate into y_vi / y_ir; on fwd tiles the
                    # D-skip (combined D_k + D_{k+2}) folds into the same op:
                    # y = xs * D + p_y
                    for (s, e, _k) in ypieces:
                        sl = slice(s - c0, e - c0)
                        if t < 2:
                            yt = t_yvi if t == 0 else t_yir
                            nc.vector.scalar_tensor_tensor(
                                yt[:, s - HW:e - HW],
                                xs_view(t, s, e - s).bitcast(F32),
                                t_dvec[:, t:t + 1], p_y[:, sl],
                                OP.mult, OP.add)
                        elif s < HW:  # t2 k2 -> vi
                            nc.vector.tensor_add(t_yvi[:, s:e], t_yvi[:, s:e],
                                                 p_y[:, sl])
                        else:         # t2 k3 -> ir
                            nc.vector.tensor_add(t_yir[:, s - HW:e - HW],
                                                 t_yir[:, s - HW:e - HW], p_y[:, sl])

        # =========== A2A: reshard y channels -> positions ===========
        for j in range(NCORES):
            nc.sync.dma_start(out=d_a2i[j, 0:DSL, :],
                              in_=t_yvi[:, j * PC:(j + 1) * PC])
            nc.sync.dma_start(out=d_a2i[j, DSL:2 * DSL, :],
                              in_=t_yir[:, j * PC:(j + 1) * PC])
        nc.gpsimd.collective_compute("AllToAll", OP.bypass, RG,
                                     ins=[d_a2i[:]], outs=[d_a2o[:]])

        # =========== PHASE C: LN + gate + out (position-sharded) ===========
        with tc.tile_pool(name="pcq", bufs=2) as pcq, \
             tc.tile_pool(name="pcp", bufs=1, space="PSUM") as pcp:
            # gather y chunks [96, PC] x (2 chunks, 2 mods)
            t_y = {}
            for mod, roff in (("vi", 0), ("ir", DSL)):
                for ck in range(2):
                    ty = pcq.tile([DM, PC], F16, tag=f"y{mod}{ck}", name=f"y{mod}{ck}")
                    for jj in range(4):
                        j = ck * 4 + jj
                        nc.sync.dma_start(out=ty[jj * DSL:(jj + 1) * DSL, :],
                                          in_=d_a2o[j, roff:roff + DSL, :])
                    t_y[(mod, ck)] = ty
            # chan-attn scales s = 1 + sigmoid(f2 @ (relu(va)+relu(vm)))
            t_vr = pcq.tile([12, 4], F32, tag="vr")
            nc.scalar.activation(t_vr[:], t_v1o[:], AF.Relu)
            t_vw = pcq.tile([12, 2], F32, tag="vw")
            nc.vector.tensor_add(t_vw[:, 0:1], t_vr[:, 0:1], t_vr[:, 1:2])
            nc.vector.tensor_add(t_vw[:, 1:2], t_vr[:, 2:3], t_vr[:, 3:4])
            t_s = {}
            for ck in range(2):
                p_ca = pcp.tile([DM, 2], F32, tag="pca")
                for mod_i in range(2):
                    nc.tensor.matmul(p_ca[:, mod_i:mod_i + 1], t_f2[:, mod_i, ck, :],
                                     t_vw[:, mod_i:mod_i + 1],
                                     start=True, stop=True)
                t_e = pcq.tile([DM, 2], F32, tag="cae")
                nc.scalar.activation(t_e[:], p_ca[:], AF.Exp, bias=0.0, scale=-1.0)
                nc.vector.tensor_scalar_add(t_e[:], t_e[:], 1.0)
                t_r = pcq.tile([DM, 2], F32, tag=f"car{ck}", name=f"car{ck}")
                nc.vector.reciprocal(t_r[:], t_e[:])          # sigmoid
                nc.vector.tensor_scalar_add(t_r[:], t_r[:], 1.0)  # 1 + sigmoid
                t_s[ck] = t_r
            # LN per modality
            t_fin = {}
            for mod in ("vi", "ir"):
                p_s1 = pcp.tile([1, PC], F32, tag="s1")
                p_s2 = pcp.tile([1, PC], F32, tag="s2")
                for ck in range(2):
                    nc.tensor.matmul(p_s1[:], t_onec[:],
                                     t_y[(mod, ck)][:], start=(ck == 0),
                                     stop=(ck == 1))
                for ck in range(2):
                    t_sq = pcq.tile([DM, PC], F16, tag="sq")
                    nc.scalar.activation(t_sq[:], t_y[(mod, ck)][:], AF.Square)
                    nc.tensor.matmul(p_s2[:], t_onec[:],
                                     t_sq[:], start=(ck == 0), stop=(ck == 1))
                t_mu = pcq.tile([1, PC], F32, tag="mu")
                nc.vector.tensor_scalar_mul(t_mu[:], p_s1[:], 1.0 / DI)
                t_musq = pcq.tile([1, PC], F32, tag="musq")
                nc.vector.tensor_mul(t_musq[:], t_mu[:], t_mu[:])
                t_var = pcq.tile([1, PC], F32, tag="var")
                nc.vector.scalar_tensor_tensor(t_var[:], p_s2[:], 1.0 / DI,
                                               t_musq[:], OP.mult, OP.subtract)
                t_eps = pcq.tile([1, 1], F32, tag="eps")
                nc.vector.memset(t_eps[:], 1e-5)
                t_lnv = pcq.tile([1, PC], F32, tag="lnv")
                nc.scalar.activation(t_lnv[:], t_var[:], AF.Ln, bias=t_eps[:], scale=1.0)
                t_rstd = pcq.tile([1, PC], R32, tag="rstd")
                nc.scalar.activation(t_rstd[:], t_lnv[:], AF.Exp, bias=0.0, scale=-0.5)
                t_mur = pcq.tile([1, PC], R32, tag="mur")
                nc.vector.tensor_mul(t_mur[:], t_mu[:], t_rstd[:].bitcast(F32))
                p_q = pcp.tile([DM, PC], F32, tag="pq")
                nc.tensor.matmul(p_q[:], t_oner[:], t_rstd[:], start=True, stop=True)
                p_m = pcp.tile([DM, PC], F32, tag="pm")
                nc.tensor.matmul(p_m[:], t_oner[:], t_mur[:], start=True, stop=True)
                gb = {"vi": (0, 1), "ir": (2, 3)}[mod]
                for ck in range(2):
                    t_t = pcq.tile([DM, PC], F32, tag="lt")
                    nc.vector.tensor_mul(t_t[:], t_y[(mod, ck)][:], p_q[:])
                    t_t2 = pcq.tile([DM, PC], F32, tag="lt2")
                    nc.vector.tensor_sub(t_t2[:], t_t[:], p_m[:])
                    t_yn = pcq.tile([DM, PC], F32, tag="yn")
                    nc.scalar.activation(t_yn[:], t_t2[:], AF.Identity,
                                         bias=t_lnw[:, ck, gb[1]:gb[1] + 1],
                                         scale=t_lnw[:, ck, gb[0]:gb[0] + 1])
                    # gate: fin += yn * z * s
                    t_m1 = pcq.tile([DM, PC], F32, tag="m1")
                    nc.vector.tensor_mul(t_m1[:], t_yn[:], t_z[(mod, ck)][:])
                    if mod == "vi":
                        t_f = pcq.tile([DM, PC], F32, tag=f"fin{ck}", name=f"fin{ck}")
                        nc.vector.tensor_scalar_mul(t_f[:], t_m1[:],
                                                    t_s[ck][:, 0:1])
                        t_fin[ck] = t_f
                    else:
                        nc.vector.scalar_tensor_tensor(t_fin[ck][:], t_m1[:],
                                                       t_s[ck][:, 1:2], t_fin[ck][:],
                                                       OP.mult, OP.add)
            p_o = pcp.tile([DM, PC], F32, tag="po")
            for ck in range(2):
                t_finr = pcq.tile([DM, PC], R32, tag=f"finr{ck}", name=f"finr{ck}")
                nc.vector.tensor_copy(t_finr[:], t_fin[ck][:])
                nc.tensor.matmul(p_o[:], t_wout[:, ck, :], t_finr[:],
                                 start=(ck == 0), stop=(ck == 1))
            t_o = pcq.tile([DM, PC], F32, tag="o")
            nc.scalar.copy(t_o[:], p_o[:])
            nc.sync.dma_start(out=o_out[:], in_=t_o[:])

    nc.finalize()
    return nc


def _prep_inputs(inputs):
    """Host-side prep: slice/transpose weights per core. Returns in_maps."""
    g = {k: np.asarray(v, dtype=np.float32) for k, v in inputs.items()}
    x_vi = g["x_vi"].reshape(HW, DM)
    x_ir = g["x_ir"].reshape(HW, DM)
    xvt = np.ascontiguousarray(x_vi.T)
    xit = np.ascontiguousarray(x_ir.T)
    A = -np.exp(g["A_logs"]).reshape(K, DI, NST)
    Ds = g["Ds"].reshape(K, DI)
    in_maps = []
    for c in range(NCORES):
        S = slice(c * DSL, (c + 1) * DSL)
        m = {}
        m["xvt"] = xvt.astype(np.float16)
        m["xit"] = xit.astype(np.float16)
        # packed in_proj lhsT: x blocks sub@0 vi@32 ir@64, z cols [zv | zi]
        wxA = np.zeros((DM, 96), np.float32)
        wxB = np.zeros((DM, 96), np.float32)
        wzA = np.zeros((DM, 48), np.float32)
        wzB = np.zeros((DM, 48), np.float32)
        wxA[:, 0:24] = g["W_sub"][S].T
        wxA[:, 32:56] = g["W_vi"][S].T
        wxB[:, 0:24] = -g["W_sub"][S].T
        wxB[:, 64:88] = g["W_ir"][S].T
        wzA[:, 0:24] = g["W_vi"][DI:][S].T
        wzB[:, 24:48] = g["W_ir"][DI:][S].T
        m["wxA"], m["wxB"] = wxA.astype(np.float16), wxB.astype(np.float16)
        m["wzA"], m["wzB"] = wzA.astype(np.float16), wzB.astype(np.float16)
        w9p = np.zeros((96, 9, 96), np.float32)
        b72p = np.zeros((96, 1), np.float32)
        for nm in ("sub", "vi", "ir"):
            mo = MODOFF[nm]
            cw = g[f"conv_w_{nm}"][S, 0]      # [DSL, 3, 3]
            for tap in range(9):
                for d in range(DSL):
                    w9p[mo + d, tap, mo + d] = cw[d, tap // 3, tap % 3]
            b72p[mo:mo + DSL, 0] = g[f"conv_b_{nm}"][S]
        m["w9p"], m["b72p"] = w9p, b72p
        # x_dbl block lhsT: 6 groups (k0s k0v k1s k1i k2v k3i) x 14 rows
        wpk84 = np.zeros((96, GR), np.float32)
        for gi2, (k, nm) in enumerate(
                ((0, "sub"), (0, "vi"), (1, "sub"), (1, "ir"),
                 (2, "vi"), (3, "ir"))):
            mo = MODOFF[nm]
            wpk84[mo:mo + DSL, gi2 * 14:(gi2 + 1) * 14] = \
                g["x_proj_weight"][k][:, S].T
        m["wpk84"] = wpk84
        wdtr = np.zeros((RNK, K, LANES), np.float32)
        dtb = np.zeros((LANES, K), np.float32)
        asc = np.zeros((LANES, K), np.float32)
        for k in range(K):
            for n in range(NST):
                for d in range(DSL):
                    lane = n * DSL + d
                    wdtr[:, k, lane] = g["dt_projs_weight"][k, c * DSL + d, :]
                    dtb[lane, k] = g["dt_projs_bias"][k, c * DSL + d]
                    asc[lane, k] = A[k, c * DSL + d, n]
        m["wdtr"] = wdtr.astype(np.float16)
        m["dtb"], m["asc"] = dtb, asc
        rep24 = np.zeros((DSL, LANES), np.float32)
        repb = np.zeros((NST, LANES), np.float32)
        m96 = np.zeros((LANES, DSL), np.float32)
        for n in range(NST):
            for d in range(DSL):
                rep24[d, n * DSL + d] = 1
                repb[n, n * DSL + d] = 1
                m96[n * DSL + d, d] = 1
        m["rep24"], m["repb"], m["m96"] = (rep24.astype(np.float16),
            repb.astype(np.float16), m96.astype(np.float16))
        dvec = np.zeros((DSL, 2), np.float32)
        dvec[:, 0] = Ds[0, S] + Ds[2, S]
        dvec[:, 1] = Ds[1, S] + Ds[3, S]
        m["dvec"] = dvec
        f1q = np.zeros((48, 2, 12), np.float32)
        f1q[0:24, 0] = g["ca_vi_f1"][:, S].T
        f1q[24:48, 1] = g["ca_ir_f1"][:, S].T
        m["f1q"] = f1q
        f2 = np.zeros((12, 2, 2, DM), np.float32)
        for ck in range(2):
            f2[:, 0, ck] = g["ca_vi_f2"][ck * DM:(ck + 1) * DM].T
            f2[:, 1, ck] = g["ca_ir_f2"][ck * DM:(ck + 1) * DM].T
        m["f2"] = f2
        lnw = np.zeros((DM, 2, 4), np.float32)
        for ck in range(2):
            cs = slice(ck * DM, (ck + 1) * DM)
            lnw[:, ck, 0] = g["ln_vi_g"][cs]
            lnw[:, ck, 1] = g["ln_vi_b"][cs]
            lnw[:, ck, 2] = g["ln_ir_g"][cs]
            lnw[:, ck, 3] = g["ln_ir_b"][cs]
        m["lnw"] = lnw
        wout = np.zeros((DM, 2, DM), np.float32)
        for ck in range(2):
            wout[:, ck] = g["W_out"][:, ck * DM:(ck + 1) * DM].T
        m["wout"] = wout
        wz = np.zeros((DM, 4, DM), np.float32)
        wz[:, 0] = g["W_vi"][DI:][0:DM].T
        wz[:, 1] = g["W_vi"][DI:][DM:DI].T
        wz[:, 2] = g["W_ir"][DI:][0:DM].T
        wz[:, 3] = g["W_ir"][DI:][DM:DI].T
        m["wz"] = wz.astype(np.float16)
        m["onec"] = np.ones((DM, 1), np.float16)
        m["oner"] = np.ones((1, DM), np.float32)
        m["xvc"] = np.ascontiguousarray(xvt[:, c * PC:(c + 1) * PC]).astype(np.float16)
        m["xic"] = np.ascontiguousarray(xit[:, c * PC:(c + 1) * PC]).astype(np.float16)
        in_maps.append(m)
    return in_maps


def kernel(**inputs):
    if "nc" not in _cache:
        _cache["nc"] = _build()
    nc = _cache["nc"]
    in_maps = _prep_inputs(inputs)
    res = run_bass_kernel_spmd(nc, in_maps, core_ids=list(range(NCORES)))
    out = np.zeros((DM, HW), np.float32)
    for c in range(NCORES):
        out[:, c * PC:(c + 1) * PC] = res.results[c]["out"]
    return out.T.reshape(B, H, W, DM).astype(np.float32)
